# revision 35
# baseline (speedup 1.0000x reference)
"""Trainium2 Bass kernel for nn_DrugRank (GNN message passing), 8 NeuronCores.

Architecture (v3 — replicated L1 transform, z-trick tail):

  - Bio branch pruned to the 2-hop in-neighborhood of node 49999 (the only
    row the reference consumes); replicated per core.
  - cll graph (3451 nodes, 55216 edges, 4 GCN layers): dst-node sharded,
    512 nodes (4 blocks of 128) per core. GCN normalization folded host-side
    into dense per-(src-chunk, dst-block) adjacency tiles Q.
  - Layer 1: h1 = x_cll @ Wc1 is collective-free (x replicated input), so
    every core computes the FULL h1 (27 chunks) locally — no AllGather and
    the one-time CC barrier (~30-50us) overlaps this compute instead of
    idling the PE.
  - Layers 1-3 aggregate via 27x4 PSUM-accumulated 128x128x200 matmuls;
    per-layer AllGather of the 200KB transformed slice (AG2, AG3 only).
  - Layer 4 via associativity: (Q^T c3) @ Wc4 = Q^T (c3 @ Wc4). Each core
    computes z = c3_own @ Wc4 [512,3], AllGathers 3KB instead of 204KB,
    then P^T = sum_s z_s^T Q_s as 27 skinny matmuls. Saves AG4's wire time
    and its dead window.
  - mol branch, bio branch, and fusion-head mini-chains are emitted into
    the AG windows. Dense-1 row-sharded + AllReduce, head replicated.
"""

import numpy as np

import concourse.bacc as bacc
import concourse.bass as bass
import concourse.mybir as mybir
import concourse.tile as tile
from concourse.bass_utils import run_bass_kernel_spmd

NCORES = 8
P = 128
F = 200

CLL_N, CLL_E, CLL_PAD, CLL_NPC = 3451, 55216, 4096, 512
CLL_NBLK = CLL_NPC // P                 # 4 dst blocks / core
CLL_NCH = 27                            # src chunks with real nodes
CLL_NF = CLL_NCH * P                    # 3456 packed src nodes
N_BIO = 50000
BIO_S2, BIO_S1 = 768, 128               # padded bio 2-hop sets
BIO_NCH = BIO_S2 // P                   # 6
MOL_N, MOL_E = 64, 128

f32 = mybir.dt.float32
f16 = mybir.dt.float16
i16 = mybir.dt.int16
RELU = mybir.ActivationFunctionType.Relu
COPY = mybir.ActivationFunctionType.Copy
EQ = mybir.AluOpType.is_equal
MUL = mybir.AluOpType.mult
ADD = mybir.AluOpType.add

# Small weights are packed into a few blob tensors (one DMA each) because
# every dma_start costs ~0.6-1.3us of engine-queue time and the HWDGE
# completion-tracking window stalls the queue after ~10 outstanding DMAs.
# Each entry: (name, rows, cols); rows are zero-padded to the blob height.
BLOB16_SPEC = [
    ("wc2a", 128, F), ("wc2b", 72, F),
    ("wc3a", 128, F), ("wc3b", 72, F),
    ("wc4a", 128, 3), ("wc4b", 72, 3),
    ("wb2a", 128, F), ("wb2b", 72, F),
    ("qb1", 128, BIO_NCH * BIO_S1), ("qb2", 128, 1),
    ("xbT0", 128, BIO_S2), ("xbT1", 128, BIO_S2),
    ("wb1_0", 128, F), ("wb1_1", 128, F),
    ("ident16", 128, 128),
]
BLOB32E_SPEC = [
    ("bc1_rep", 128, F), ("bc2_rep", 128, F), ("bc3_rep", 128, F),
    ("bc4_rep", 128, 4),
    ("iota32", 128, 128), ("ident32", 128, 128), ("ones32", 128, 1),
    ("mol_slot", 128, 1),
]
BLOB32A_SPEC = [
    ("xmolT", 64, MOL_N),
    ("wm1r", 64, F), ("wm1s", 64, F),
    ("bm1_rep", 64, F), ("bm2_rep", 64, F),
]
BLOB32B_SPEC = [
    ("bb1_rep", 128, F),
    ("wm2ra", 128, F), ("wm2rb", 72, F),
    ("wm2sa", 128, F), ("wm2sb", 72, F),
    ("wlma", 128, 128), ("wlmb", 72, 128),
    ("wlba", 128, 128), ("wlbb", 72, 128),
    ("wd1_0", 128, 500), ("wd1_1", 128, 500),
    ("wd2_0", 125, 256), ("wd2_1", 125, 256),
    ("wd2_2", 125, 256), ("wd2_3", 125, 256),
    ("bd1_t", 125, 4), ("bd2_t", 128, 2),
    ("blm_col", 128, 1), ("blb_col", 128, 1),
    ("bl1c_pk", 128, 8), ("wcat2_pk", 128, 8),
]
BROW_SPEC = [
    ("bb2_row", 1, F), ("bl2c_row", 1, 1024), ("bl3c_row", 1, 256),
    ("bcat1_row", 1, 1024), ("bcat2_t", 1, 1),
]


def _blob_offsets(spec):
    offs, c = {}, 0
    for name, rows, cols in spec:
        offs[name] = (c, rows, cols)
        c += cols
    return offs, c


BLOB16_OFF, BLOB16_N = _blob_offsets(BLOB16_SPEC)
BLOB32E_OFF, BLOB32E_N = _blob_offsets(BLOB32E_SPEC)
BLOB32A_OFF, BLOB32A_N = _blob_offsets(BLOB32A_SPEC)
BLOB32B_OFF, BLOB32B_N = _blob_offsets(BLOB32B_SPEC)
BROW_OFF, BROW_N = _blob_offsets(BROW_SPEC)
_BLOBS = [("blob16", BLOB16_OFF), ("blob32e", BLOB32E_OFF),
          ("blob32a", BLOB32A_OFF), ("blob32b", BLOB32B_OFF),
          ("brow", BROW_OFF)]


def _pack_blob(spec, parts, height, dt):
    offs, total = _blob_offsets(spec)
    blob = np.zeros((height, total), dt)
    for name, rows, cols in spec:
        a = np.asarray(parts[name])
        assert a.shape == (rows, cols), (name, a.shape, rows, cols)
        blob[:rows, offs[name][0]:offs[name][0] + cols] = a
    return np.ascontiguousarray(blob)


def blob_get(m, name):
    """Extract an unpadded sub-array from the packed blobs (test helper)."""
    for key, offs in _BLOBS:
        if name in offs:
            c0, rows, cols = offs[name]
            return np.asarray(m[key])[0:rows, c0:c0 + cols]
    raise KeyError(name)


class Slab:
    """Column window of a blob tile, sliceable like a standalone tile."""

    def __init__(self, tile, off, rows, cols):
        self.t, self.off, self.rows, self.cols = tile, off, rows, cols

    def __getitem__(self, key):
        if not isinstance(key, tuple):
            key = (key, slice(None))
        rs, cs = key
        r0 = rs.start if rs.start is not None else 0
        r1 = rs.stop if rs.stop is not None else self.rows
        c0 = cs.start if cs.start is not None else 0
        c1 = cs.stop if cs.stop is not None else self.cols
        return self.t[r0:r1, self.off + c0:self.off + c1]


# ---------------------------------------------------------------- host prep

def _pack_idx16(flat):
    n = len(flat)
    a16 = np.asarray(flat, np.int16).reshape(n // 16, 16).T
    return np.ascontiguousarray(np.tile(a16, (8, 1)))


def _pack_slots(flat, dtype=np.float16):
    n = len(flat)
    return np.ascontiguousarray(
        np.asarray(flat, np.float64).astype(dtype).reshape(n // P, P).T)


def _col(v):
    return np.ascontiguousarray(np.asarray(v, np.float32).reshape(-1, 1))


def _rep(v, rows=P):
    return np.ascontiguousarray(
        np.tile(np.asarray(v, np.float32).reshape(1, -1), (rows, 1)))


def _btile(v, p, n):
    return np.ascontiguousarray(np.asarray(v, np.float32).reshape(n, p).T)


def _rowpad(v, n):
    """[m] -> [1, n] zero-padded row."""
    v = np.asarray(v, np.float32).reshape(-1)
    o = np.zeros((1, n), np.float32)
    o[0, :len(v)] = v
    return o


def _rowpack(w, rows_pad, cols, dt=np.float16):
    """[m, cols] -> [128, (rows_pad//128)*cols]: [p, j*cols+q] = w[j*128+p, q]."""
    w = np.asarray(w, np.float32)
    wp = np.zeros((rows_pad, cols), np.float32)
    wp[:w.shape[0]] = w
    nj = rows_pad // P
    return np.ascontiguousarray(
        wp.reshape(nj, P, cols).transpose(1, 0, 2).reshape(P, nj * cols)
    ).astype(dt)


def _cll_q(edge, dinv):
    """Dense normalized adjacency, [CLL_PAD, CLL_PAD] f32."""
    src = edge[0].astype(np.int64)
    dst = edge[1].astype(np.int64)
    q = np.zeros((CLL_PAD, CLL_PAD), np.float32)
    np.add.at(q, (src, dst), (dinv[src] * dinv[dst]).astype(np.float32))
    di = np.arange(CLL_N)
    q[di, di] += (dinv[:CLL_N] * dinv[:CLL_N]).astype(np.float32)
    return q


def _bio_prune(edge, x_bio):
    """2-hop in-neighborhood of node N_BIO-1 -> (xbT_sel, Qb1_pk, Qb2)."""
    src = edge[0].astype(np.int64)
    dst = edge[1].astype(np.int64)
    deg = np.bincount(dst, minlength=N_BIO).astype(np.float64) + 1.0
    dinv = 1.0 / np.sqrt(deg)
    tgt = N_BIO - 1

    m2 = dst == tgt
    s1 = np.unique(np.concatenate([src[m2], [tgt]]))
    assert len(s1) <= BIO_S1, len(s1)
    pos1 = np.full(N_BIO, -1, np.int64)
    pos1[s1] = np.arange(len(s1))

    m1 = pos1[dst] >= 0
    e1s, e1d = src[m1], dst[m1]
    s2 = np.unique(np.concatenate([e1s, s1]))
    assert len(s2) <= BIO_S2, len(s2)
    pos2 = np.full(N_BIO, -1, np.int64)
    pos2[s2] = np.arange(len(s2))

    q1 = np.zeros((BIO_S2, BIO_S1), np.float32)
    np.add.at(q1, (pos2[e1s], pos1[e1d]),
              (dinv[e1s] * dinv[e1d]).astype(np.float32))
    q1[pos2[s1], pos1[s1]] += (dinv[s1] * dinv[s1]).astype(np.float32)

    q2 = np.zeros((BIO_S1, 1), np.float32)
    np.add.at(q2, (pos1[src[m2]], 0),
              (dinv[src[m2]] * dinv[tgt]).astype(np.float32))
    q2[pos1[tgt], 0] += np.float32(dinv[tgt] * dinv[tgt])

    xsel = np.zeros((BIO_S2, 256), np.float32)
    xsel[:len(s2)] = x_bio[s2]
    xbT = np.ascontiguousarray(xsel.T).astype(np.float16)      # [256, 768]
    # Qb1 packed [128, 6*128]: [p, s*128+d] = q1[s*128+p, d]
    q1pk = np.ascontiguousarray(
        q1.reshape(BIO_NCH, P, BIO_S1).transpose(1, 0, 2)
        .reshape(P, BIO_NCH * BIO_S1)).astype(np.float16)
    return xbT, q1pk, q2.astype(np.float16)


def prep_inputs(inp):
    meta = {}
    # ---- cll Q tiles ----
    dst = inp["edge_cll"][1].astype(np.int64)
    deg = np.bincount(dst, minlength=CLL_N).astype(np.float64) + 1.0
    dinv = np.zeros(CLL_PAD, np.float64)
    dinv[:CLL_N] = 1.0 / np.sqrt(deg)
    q = _cll_q(inp["edge_cll"], dinv)

    xcT = np.zeros((512, CLL_NF), np.float32)
    xcT[:, :CLL_N] = inp["x_cll"].T
    # full-graph xcllT packed [128, 4*3456]: [p, k*3456+n] = x_cll.T[k*128+p, n]
    xcllT_full = np.ascontiguousarray(
        xcT.reshape(4, P, CLL_NF).transpose(1, 0, 2)
        .reshape(P, 4 * CLL_NF)).astype(np.float16)

    # W1c regrouped: rows (node*3+ch) -> per core [128, 12*1000] f16,
    # col-block j = ch*4+blk, rows = local node p of that block.
    w1c = np.asarray(inp["Wl1c"], np.float32)                  # [10353, 1000]
    w1c_n = np.zeros((CLL_PAD, 3, 1000), np.float32)
    w1c_n[:CLL_N] = w1c.reshape(CLL_N, 3, 1000)

    xbT_sel, q1pk, q2 = _bio_prune(inp["edge_bio"], np.asarray(inp["x_bio"]))

    mol_s = inp["edge_mol"][0].astype(np.int64)
    mol_d = inp["edge_mol"][1].astype(np.int64)
    order = np.argsort(mol_d, kind="stable")
    mol_idx = _pack_idx16(mol_s[order])
    mol_slot = _pack_slots(mol_d[order].astype(np.float64), np.float32)

    iota = np.tile(np.arange(P, dtype=np.float32), (P, 1))
    ident = np.eye(P, dtype=np.float32)

    wc1 = np.asarray(inp["Wc1"], np.float32)                   # [512, 200]

    def _f16(x):
        return np.asarray(x, np.float16)

    def _f32(x):
        return np.asarray(x, np.float32)

    wb2 = _f16(inp["Wb2"])
    wm2r, wm2s = _f32(inp["Wm2r"]), _f32(inp["Wm2s"])
    wlm, wlb = _f32(inp["Wlm"]), _f32(inp["Wlb"])
    wd1, wd2 = _f32(inp["Wd1"]), _f32(inp["Wd2"])
    p16 = {
        "wc2a": _f16(inp["Wc2"])[:128], "wc2b": _f16(inp["Wc2"])[128:],
        "wc3a": _f16(inp["Wc3"])[:128], "wc3b": _f16(inp["Wc3"])[128:],
        "wc4a": _f16(inp["Wc4"])[:128], "wc4b": _f16(inp["Wc4"])[128:],
        "wb2a": wb2[:128], "wb2b": wb2[128:],
        "qb1": q1pk, "qb2": np.tile(q2, (1, 1)),
        "xbT0": xbT_sel[:128], "xbT1": xbT_sel[128:],
        "wb1_0": _f16(inp["Wb1"])[:128], "wb1_1": _f16(inp["Wb1"])[128:],
        "ident16": ident.astype(np.float16),
    }
    p32e = {
        "bc1_rep": _rep(inp["bc1"]), "bc2_rep": _rep(inp["bc2"]),
        "bc3_rep": _rep(inp["bc3"]),
        "bc4_rep": np.pad(_rep(inp["bc4"]), ((0, 0), (0, 1))),
        "iota32": iota, "ident32": ident,
        "ones32": np.ones((P, 1), np.float32),
        "mol_slot": mol_slot,
    }
    p32a = {
        "xmolT": np.ascontiguousarray(inp["x_mol"].T.astype(np.float32)),
        "wm1r": _f32(inp["Wm1r"]), "wm1s": _f32(inp["Wm1s"]),
        "bm1_rep": _rep(inp["bm1"], 64), "bm2_rep": _rep(inp["bm2"], 64),
    }
    p32b = {
        "bb1_rep": _rep(inp["bb1"]),
        "wm2ra": wm2r[:128], "wm2rb": wm2r[128:],
        "wm2sa": wm2s[:128], "wm2sb": wm2s[128:],
        "wlma": wlm[:128], "wlmb": wlm[128:],
        "wlba": wlb[:128], "wlbb": wlb[128:],
        "wd1_0": wd1[:128], "wd1_1": wd1[128:],
        "wd2_0": wd2[0:125], "wd2_1": wd2[125:250],
        "wd2_2": wd2[250:375], "wd2_3": wd2[375:500],
        "bd1_t": _btile(inp["bd1"], 125, 4),
        "bd2_t": _btile(inp["bd2"], 128, 2),
        "blm_col": _col(inp["blm"]), "blb_col": _col(inp["blb"]),
        "bl1c_pk": np.ascontiguousarray(
            _rowpad(inp["bl1c"], 1024).reshape(8, P).T),
        "wcat2_pk": _rowpack(inp["Wcat2"], 1024, 1, np.float32),
    }
    prow = {
        "bb2_row": np.ascontiguousarray(
            np.asarray(inp["bb2"], np.float32).reshape(1, -1)),
        "bl2c_row": _rowpad(inp["bl2c"], 1024),
        "bl3c_row": _rowpad(inp["bl3c"], 256),
        "bcat1_row": _rowpad(inp["bcat1"], 1024),
        "bcat2_t": np.asarray(inp["bcat2"], np.float32).reshape(1, 1),
    }
    shared = {
        "xcllT": xcllT_full,
        "Wc1": np.ascontiguousarray(
            wc1.reshape(4, P, F).transpose(1, 0, 2)
            .reshape(P, 4 * F)).astype(np.float16),
        "x_mol": np.asarray(inp["x_mol"], np.float32),
        "mol_idx": mol_idx,
        "Wcat1_pk": _rowpack(inp["Wcat1"], 512, 1000),
        "Wl2c_pk": _rowpack(inp["Wl2c"], 1024, 1000),
        "Wl3c_pk": _rowpack(inp["Wl3c"], 1024, 256),
        "blob16": _pack_blob(BLOB16_SPEC, p16, P, np.float16),
        "blob32e": _pack_blob(BLOB32E_SPEC, p32e, P, np.float32),
        "blob32a": _pack_blob(BLOB32A_SPEC, p32a, P, np.float32),
        "blob32b": _pack_blob(BLOB32B_SPEC, p32b, P, np.float32),
        "brow": _pack_blob(BROW_SPEC, prow, 1, np.float32),
    }
    in_maps = []
    for c in range(NCORES):
        m = dict(shared)
        lo = c * CLL_NPC
        # Qt packed [128, 27*4*128]: [p, (s*4+b)*128+d] = q[s*128+p, lo+b*128+d]
        qc = q[:CLL_NF, lo:lo + CLL_NPC]
        m["Qt"] = np.ascontiguousarray(
            qc.reshape(CLL_NCH, P, CLL_NBLK, P).transpose(1, 0, 2, 3)
            .reshape(P, CLL_NCH * CLL_NBLK * P)).astype(np.float16)
        # W1ct [128, 12*1000]: [p, (ch*4+blk)*1000+q] = w1c_n[lo+blk*128+p, ch, q]
        wslice = w1c_n[lo:lo + CLL_NPC]                         # [512, 3, 1000]
        m["W1ct"] = np.ascontiguousarray(
            wslice.reshape(CLL_NBLK, P, 3, 1000).transpose(1, 2, 0, 3)
            .reshape(P, 12 * 1000)).astype(np.float16)
        in_maps.append(m)
    return in_maps, meta


# ------------------------------------------------------------ device program

RG = [list(range(NCORES))]


def _declare_inputs(nc):
    spec = {
        "xcllT": ([P, 4 * CLL_NF], f16),
        "Qt": ([P, CLL_NCH * CLL_NBLK * P], f16),
        "Wc1": ([P, 4 * F], f16),
        "W1ct": ([P, 12 * 1000], f16),
        "x_mol": ([MOL_N, 64], f32),
        "mol_idx": ([P, 8], i16),
        "Wcat1_pk": ([P, 4 * 1000], f16),
        "Wl2c_pk": ([P, 8 * 1000], f16),
        "Wl3c_pk": ([P, 8 * 256], f16),
        "blob16": ([P, BLOB16_N], f16),
        "blob32e": ([P, BLOB32E_N], f32),
        "blob32a": ([P, BLOB32A_N], f32),
        "blob32b": ([P, BLOB32B_N], f32),
        "brow": ([1, BROW_N], f32),
    }
    return {k: nc.dram_tensor(k, s, d, kind="ExternalInput")
            for k, (s, d) in spec.items()}


def build_program(meta=None, repeat=1):
    nc = bacc.Bacc("TRN2", target_bir_lowering=False, debug=False,
                   enable_asserts=False, num_devices=NCORES,
                   num_swdge_queues=4)
    io = _declare_inputs(nc)
    out = nc.dram_tensor("out", [1, 1], f32, kind="ExternalOutput")

    # h_slice/h_full for layers 2 and 3 (AG2, AG3, each split in 2 halves);
    # z for the layer-4 trick
    h_slice = [nc.dram_tensor(f"h{l}_slice", [CLL_NPC, F], f16,
                              kind="Internal") for l in range(2)]
    h_full = [[nc.dram_tensor(f"h{l}_full{h}", [CLL_PAD // 2, F], f16,
                              kind="Internal", addr_space="Shared")
               for h in range(2)] for l in range(2)]
    z_slice = nc.dram_tensor("z_slice", [CLL_NPC, 3], f16, kind="Internal")
    z_full = nc.dram_tensor("z_full", [CLL_PAD, 3], f16, kind="Internal",
                            addr_space="Shared")
    m1_dram = nc.dram_tensor("m1_dram", [MOL_N, 256], f32, kind="Internal")
    ar_in = nc.dram_tensor("ar_in", [1024], f32, kind="Internal")
    ar_out = nc.dram_tensor("ar_out", [1024], f32, kind="Internal",
                            addr_space="Shared")

    with tile.TileContext(nc) as tc:
        for _ in range(repeat):
            _build(nc, tc, io, out, h_slice, h_full, z_slice, z_full,
                   m1_dram, ar_in, ar_out)
    nc.compile()
    return nc


def _build(nc, tc, io, out, h_slice, h_full, z_slice, z_full,
           m1_dram, ar_in, ar_out):
    with (
        tc.tile_pool(name="const", bufs=1) as cp,
        tc.tile_pool(name="wp", bufs=1) as wp,
        tc.tile_pool(name="hp", bufs=2) as hp,
        tc.tile_pool(name="sb", bufs=2) as sb,
        tc.tile_pool(name="ct", bufs=1) as ctp,
        tc.tile_pool(name="psA", bufs=1, space="PSUM") as psA,
        tc.tile_pool(name="psT", bufs=2, space="PSUM") as psT,
        tc.tile_pool(name="psM", bufs=2, space="PSUM") as psM,
    ):
        def load(pool, name, rows=None, cols=None, tag=None, dt=None,
                 eng=None):
            src = io[name]
            r = rows if rows is not None else src.shape[0]
            c = cols if cols is not None else src.shape[1]
            t = pool.tile([r, c], dt or src.dtype, tag=tag or name)
            (eng or nc.sync).dma_start(t[:], src[0:r, 0:c])
            return t

        def load2(name, tag, rows=F, cols=F, eng=None):
            """[rows>128, cols] -> two tiles [128, cols] + [rows-128, cols]."""
            a = load(wp, name, rows=P, cols=cols, tag=tag + "a", eng=eng)
            b = wp.tile([P, cols], io[name].dtype, tag=tag + "b")
            (eng or nc.sync).dma_start(b[0:rows - P, :], io[name][P:rows, 0:cols])
            return a, b

        def loadrows(name, nparts, cols, tag, rows=P, eng=None):
            """Tall [nparts*rows?, cols] tensor -> list of [128, cols] tiles."""
            ts = []
            for k in range(nparts):
                t = wp.tile([rows, cols], io[name].dtype, tag=f"{tag}{k}")
                (eng or nc.sync).dma_start(
                    t[:], io[name][k * rows:(k + 1) * rows, 0:cols])
                ts.append(t)
            return ts

        # ---- phase A inputs first: Wc1 + xcllT (8 half-chunk tiles over
        # the two HWDGE queues, first halves first), Qt pieces on SWDGE ----
        wc1 = load(wp, "Wc1", eng=nc.scalar)       # [128, 4*200] f16, k-major
        b32e = load(wp, "blob32e", eng=nc.scalar)  # early-critical constants

        PIECES = [(0, 7), (7, 14), (14, 21), (21, CLL_NCH)]

        # xcllT tiled per (k, piece) and Qt per piece, all loads round-robin
        # over the 3 DMA queues in CONSUMPTION order, so the per-queue
        # ~100GB/s arrival front-runs the fused transform+aggregate loop.
        ENGS = [nc.sync, nc.scalar, nc.gpsimd]
        xckp = [[wp.tile([P, (s1 - s0) * P], f16, tag=f"xc{k}_{pi}",
                         name=f"xc{k}_{pi}")
                 for pi, (s0, s1) in enumerate(PIECES)] for k in range(4)]
        qt_p = [wp.tile([P, (s1 - s0) * CLL_NBLK * P], f16, tag=f"Qt{pi}",
                        name=f"Qt{pi}")
                for pi, (s0, s1) in enumerate(PIECES)]
        ei = 0
        for pi, (s0, s1) in enumerate(PIECES):
            for k in range(4):
                ENGS[ei % 3].dma_start(
                    xckp[k][pi][:],
                    io["xcllT"][:, k * CLL_NF + s0 * P:k * CLL_NF + s1 * P])
                ei += 1
            ENGS[ei % 3].dma_start(
                qt_p[pi][:],
                io["Qt"][:, s0 * CLL_NBLK * P:s1 * CLL_NBLK * P])
            ei += 1

        def piece_of(s):
            return next(i for i, (a, b) in enumerate(PIECES) if a <= s < b)

        def qtcol(s, b):
            pi = piece_of(s)
            c0 = ((s - PIECES[pi][0]) * CLL_NBLK + b) * P
            return qt_p[pi][:, c0:c0 + P]

        # ---- phase A fused with L1 aggregation: per piece, transform
        # h1 chunks then immediately aggregate them, so the agg matmuls of
        # piece p hide the xcllT DMA tail for piece p+1 ----
        h1p = []
        for pi, (s0, s1) in enumerate(PIECES):
            h1p.append(hp.tile([P, s1 - s0, F], f16, tag=f"hft{pi}",
                               name=f"hft{pi}"))
        h1pss = [psA.tile([P, F], f32, tag=f"agg{b}", name=f"agg{b}",
                          space="PSUM") for b in range(CLL_NBLK)]
        for pi, (s0, s1) in enumerate(PIECES):
            for s in range(s0, s1):
                ps = psM.tile([P, F], f32, tag="m", space="PSUM")
                for k in range(4):
                    nc.tensor.matmul(
                        ps[:], xckp[k][pi][:, (s - s0) * P:(s - s0 + 1) * P],
                        wc1[:, k * F:(k + 1) * F],
                        start=(k == 0), stop=(k == 3))
                nc.vector.tensor_copy(h1p[pi][:, s - s0, :], ps[:])
            for s in range(s0, s1):
                for b in range(CLL_NBLK):
                    nc.tensor.matmul(h1pss[b][:], qtcol(s, b),
                                     h1p[pi][:, s - s0, 0:F],
                                     start=(s == 0), stop=(s == CLL_NCH - 1))

        # blob loads: one DMA each (trigger cost + the HWDGE completion
        # window make many small DMAs poisonous), then early mol prep
        mol_idx_sb = load(cp, "mol_idx", eng=nc.scalar)
        b16 = load(wp, "blob16", eng=nc.scalar)
        browt = load(wp, "brow", eng=nc.scalar)
        b32a = load(wp, "blob32a", eng=nc.sync)
        b32b = load(wp, "blob32b", eng=nc.sync)

        def _slab(tile, offs, name):
            c0, r, c = offs[name]
            return Slab(tile, c0, r, c)

        def S16(n):
            return _slab(b16, BLOB16_OFF, n)

        def S32a(n):
            return _slab(b32a, BLOB32A_OFF, n)

        def S32e(n):
            return _slab(b32e, BLOB32E_OFF, n)

        def S32b(n):
            return _slab(b32b, BLOB32B_OFF, n)

        def SR(n):
            return _slab(browt, BROW_OFF, n)

        iota32 = S32e("iota32")
        ident32 = S32e("ident32")
        ident16 = S16("ident16")
        ones32 = S32e("ones32")
        mol_slot_sb = S32e("mol_slot")
        xmolT_sb = S32a("xmolT")
        v1 = sb.tile([P, 1, 64], f32, tag="vm")
        nc.gpsimd.dma_gather(v1[:], io["x_mol"].ap(), mol_idx_sb[:],
                             MOL_E, MOL_E, 64)

        wc2 = (S16("wc2a"), S16("wc2b"))
        bc_rep = [S32e("bc1_rep"), S32e("bc2_rep"), S32e("bc3_rep")]

        def allgather(src, dst):
            nc.gpsimd.collective_compute(
                "AllGather", mybir.AluOpType.bypass, replica_groups=RG,
                ins=[src], outs=[dst])

        def hload2(hAB):
            """Split-AG halves hA/hB [2048, F] -> chunk_srcs list of
            (tile, idx, s) in processing order (half A chunks first)."""
            srcs = []
            for half, hx in enumerate(hAB):
                src = hx.ap().rearrange("(c p) f -> p c f", p=P)
                for sub in range(2):
                    t = hp.tile([P, 8, F], f16, tag=f"hft{half * 2 + sub}",
                                name=f"hft{half * 2 + sub}")
                    nc.sync.dma_start(t[:], src[:, sub * 8:(sub + 1) * 8, :])
                    for c in range(sub * 4, sub * 4 + 4):
                        for b in range(2):
                            s = 4 * c + 2 * half + b
                            if s < CLL_NCH:
                                srcs.append((t, (c - sub * 4) * 2 + b, s))
            return srcs

        def transpose_to(src_sb, dst0, dst1, bcol):
            """src [128, 200] f32 -> dst0[128, bcol:+128], dst1[72, bcol:+128] f16."""
            pt = psT.tile([P, P], f32, tag="tp", space="PSUM")
            nc.tensor.transpose(pt[0:P, 0:P], src_sb[:, 0:P], ident32[:])
            nc.vector.tensor_copy(dst0[:, bcol:bcol + P], pt[0:P, 0:P])
            pt2 = psT.tile([P, P], f32, tag="tp", space="PSUM")
            nc.tensor.transpose(pt2[0:F - P, 0:P], src_sb[:, P:F], ident32[:])
            nc.vector.tensor_copy(dst1[0:F - P, bcol:bcol + P],
                                  pt2[0:F - P, 0:P])

        def agg_blocks(chunk_srcs):
            """4 PSUM accumulators over an arbitrary chunk processing order
            (starts as soon as the first source tile is available)."""
            pss = [psA.tile([P, F], f32, tag=f"agg{b}", name=f"agg{b}",
                            space="PSUM") for b in range(CLL_NBLK)]
            n = len(chunk_srcs)
            for pos, (t, idx, s) in enumerate(chunk_srcs):
                for b in range(CLL_NBLK):
                    nc.tensor.matmul(pss[b][:], qtcol(s, b),
                                     t[:, idx, 0:F],
                                     start=(pos == 0), stop=(pos == n - 1))
            return pss

        def layer_tail(pss, wnext, brep, dst_dram, ncols=F, ag_halves=None):
            """relu(+bias), transform by wnext ([128,c]+[72,c] tiles), store
            [512, ncols] f16 slices; optionally trigger the half-AllGathers
            after blocks 1 and 3."""
            cT0 = ctp.tile([P, CLL_NPC], f16, tag="cT0")
            cT1 = ctp.tile([P, CLL_NPC], f16, tag="cT1")
            for b in range(CLL_NBLK):
                t2 = sb.tile([P, F], f32, tag="ev1")
                nc.vector.tensor_tensor(t2[:], pss[b][:], brep[:], op=ADD)
                cblk = sb.tile([P, F], f32, tag="cblk", bufs=3)
                nc.scalar.activation(cblk[:], t2[:], RELU)
                transpose_to(cblk, cT0, cT1, b * P)
                wa, wb_ = wnext
                ph = psM.tile([P, ncols], f32, tag="m", space="PSUM")
                nc.tensor.matmul(ph[:], cT0[:, b * P:(b + 1) * P],
                                 wa[:, 0:ncols], start=True, stop=False)
                nc.tensor.matmul(ph[:], cT1[0:F - P, b * P:(b + 1) * P],
                                 wb_[0:F - P, 0:ncols], start=False, stop=True)
                hst = sb.tile([P, ncols], f16, tag="hst", bufs=3)
                nc.vector.tensor_copy(hst[:], ph[:])
                nc.sync.dma_start(dst_dram[b * P:(b + 1) * P, 0:ncols],
                                  hst[:])
                if ag_halves is not None and b % 2 == 1:
                    half = b // 2
                    allgather(dst_dram[half * 2 * P:(half * 2 + 2) * P,
                                       0:ncols],
                              ag_halves[half].ap())

        # ---- layer 1 tail -> h2 -> AG2 ----
        layer_tail(h1pss, wc2, bc_rep[0], h_slice[0], ag_halves=h_full[0])

        # ---- bio mini-branch (fills the AG2/barrier window) ----
        xbT = [S16("xbT0"), S16("xbT1")]
        qb1 = S16("qb1")
        qb2 = S16("qb2")
        wb1 = [S16("wb1_0"), S16("wb1_1")]
        wb2 = (S16("wb2a"), S16("wb2b"))
        bb1r = S32b("bb1_rep")
        bb2row = SR("bb2_row")
        wc3 = (S16("wc3a"), S16("wc3b"))
        wc4 = (S16("wc4a"), S16("wc4b"))
        bc4r = S32e("bc4_rep")
        h1b = sb.tile([P, BIO_NCH, F], f16, tag="h1b", bufs=1)
        for j in range(BIO_NCH):
            ps = psM.tile([P, F], f32, tag="m", space="PSUM")
            for k in range(2):
                nc.tensor.matmul(ps[:], xbT[k][:, j * P:(j + 1) * P],
                                 wb1[k][:],
                                 start=(k == 0), stop=(k == 1))
            nc.vector.tensor_copy(h1b[:, j, :], ps[:])
        psb = psM.tile([P, F], f32, tag="m", space="PSUM")
        for j in range(BIO_NCH):
            nc.tensor.matmul(psb[:], qb1[:, j * P:(j + 1) * P], h1b[:, j, 0:F],
                             start=(j == 0), stop=(j == BIO_NCH - 1))
        tb1 = sb.tile([P, F], f32, tag="ev1")
        nc.vector.tensor_tensor(tb1[:], psb[:], bb1r[:], op=ADD)
        c1b = sb.tile([P, F], f32, tag="c1b", bufs=1)
        nc.scalar.activation(c1b[:], tb1[:], RELU)
        c1bT0 = sb.tile([P, P], f16, tag="c1bT0", bufs=1)
        c1bT1 = sb.tile([P, P], f16, tag="c1bT1", bufs=1)
        ptb = psT.tile([P, P], f32, tag="tp", space="PSUM")
        nc.tensor.transpose(ptb[0:P, 0:P], c1b[:, 0:P], ident32[:])
        nc.vector.tensor_copy(c1bT0[:], ptb[0:P, 0:P])
        ptb2 = psT.tile([P, P], f32, tag="tp", space="PSUM")
        nc.tensor.transpose(ptb2[0:F - P, 0:P], c1b[:, P:F], ident32[:])
        nc.vector.tensor_copy(c1bT1[0:F - P, :], ptb2[0:F - P, 0:P])
        ph2 = psM.tile([P, F], f32, tag="m", space="PSUM")
        nc.tensor.matmul(ph2[:], c1bT0[:, 0:P], wb2[0][:],
                         start=True, stop=False)
        nc.tensor.matmul(ph2[:], c1bT1[0:F - P, 0:P], wb2[1][0:F - P, :],
                         start=False, stop=True)
        h2b = sb.tile([P, F], f16, tag="h2b", bufs=1)
        nc.vector.tensor_copy(h2b[:], ph2[:])
        pr = psM.tile([1, F], f32, tag="m", space="PSUM")
        nc.tensor.matmul(pr[:], qb2[:], h2b[:], start=True, stop=True)
        tb2 = sb.tile([1, F], f32, tag="ev1")
        nc.vector.tensor_tensor(tb2[0:1, :], pr[0:1, :], bb2row[0:1, :], op=ADD)
        brow = sb.tile([1, F], f32, tag="brow", bufs=1)
        nc.scalar.activation(brow[0:1, :], tb2[0:1, :], RELU)
        # bvec column [200, 1] for the head
        bgc0 = sb.tile([P, 1], f32, tag="bgc0", bufs=1)
        bgc1 = sb.tile([P, 1], f32, tag="bgc1", bufs=1)
        prc = psT.tile([P, P], f32, tag="tp", space="PSUM")
        nc.tensor.transpose(prc[0:P, 0:1], brow[0:1, 0:P], ident32[0:1, 0:1])
        nc.vector.tensor_copy(bgc0[:], prc[0:P, 0:1])
        prc2 = psT.tile([P, P], f32, tag="tp", space="PSUM")
        nc.tensor.transpose(prc2[0:F - P, 0:1], brow[0:1, P:F],
                            ident32[0:1, 0:1])
        nc.vector.tensor_copy(bgc1[0:F - P, :], prc2[0:F - P, 0:1])

        # head weights (slabs) + big packed tables
        wlb = (S32b("wlba"), S32b("wlbb"))
        blb = S32b("blb_col")
        wd1 = (S32b("wd1_0"), S32b("wd1_1"))
        bd1 = S32b("bd1_t")
        wd2t = [S32b(f"wd2_{k}") for k in range(4)]
        bd2 = S32b("bd2_t")
        bl1c = S32b("bl1c_pk")
        bl2c = SR("bl2c_row")
        wl2cpk = load(wp, "Wl2c_pk", eng=nc.sync)

        # ---- mol branch part 1 + gather-2 trigger (SWDGE latency hides
        # under the AG2 window) ----
        wm1r = S32a("wm1r")
        wm1s = S32a("wm1s")
        bm1r = S32a("bm1_rep")
        bm2r = S32a("bm2_rep")
        mM = sb.tile([P, 64], f32, tag="Mmol", bufs=1)
        nc.vector.tensor_scalar(mM[:], iota32[:, 0:64], mol_slot_sb[:, 0:1],
                                None, op0=EQ)
        agg_ps = psM.tile([64, 64], f32, tag="m", space="PSUM")
        nc.tensor.matmul(agg_ps[:], mM[:], v1[:, 0, :], start=True, stop=True)
        agg_sb = sb.tile([64, 64], f32, tag="mol1")
        nc.vector.tensor_copy(agg_sb[:], agg_ps[:])
        pt = psT.tile([P, P], f32, tag="tp", space="PSUM")
        nc.tensor.transpose(pt[0:64, 0:64], agg_sb[0:64, 0:64],
                            ident32[0:64, 0:64])
        aggT = sb.tile([64, 64], f32, tag="mol2")
        nc.vector.tensor_copy(aggT[:], pt[0:64, 0:64])
        h1_ps = psM.tile([64, F], f32, tag="m", space="PSUM")
        nc.tensor.matmul(h1_ps[:], aggT[:], wm1r[:], start=True, stop=False)
        nc.tensor.matmul(h1_ps[:], xmolT_sb[:], wm1s[:], start=False, stop=True)
        t_m1 = sb.tile([64, F], f32, tag="mol3")
        nc.vector.tensor_tensor(t_m1[:], h1_ps[:], bm1r[0:64, :], op=ADD)
        m1_sb = sb.tile([64, F], f32, tag="mol4", bufs=1)
        nc.scalar.activation(m1_sb[:], t_m1[:], RELU)
        nc.gpsimd.dma_start(m1_dram[0:64, 0:F], m1_sb[:])

        wm2r = (S32b("wm2ra"), S32b("wm2rb"))
        wm2s = (S32b("wm2sa"), S32b("wm2sb"))
        v2 = sb.tile([P, 1, 256], f32, tag="vm2")
        nc.gpsimd.dma_gather(v2[:], m1_dram.ap(), mol_idx_sb[:],
                             MOL_E, MOL_E, 256)

        # ---- mol branch part 2 (fills the AG2 window) ----
        agg2_ps = psM.tile([64, F], f32, tag="m", space="PSUM")
        nc.tensor.matmul(agg2_ps[:], mM[:], v2[:, 0, 0:F], start=True, stop=True)
        agg2_sb = sb.tile([64, F], f32, tag="mol1")
        nc.vector.tensor_copy(agg2_sb[:], agg2_ps[:])
        a2T0 = sb.tile([P, 64], f32, tag="mol5")
        a2T1 = sb.tile([P, 64], f32, tag="mol6")
        m1T0 = sb.tile([P, 64], f32, tag="mol7")
        m1T1 = sb.tile([P, 64], f32, tag="mol8")
        for srcT, d0, d1 in ((agg2_sb, a2T0, a2T1), (m1_sb, m1T0, m1T1)):
            pt1 = psT.tile([P, P], f32, tag="tp", space="PSUM")
            nc.tensor.transpose(pt1[0:P, 0:64], srcT[0:64, 0:P],
                                ident32[0:64, 0:64])
            nc.vector.tensor_copy(d0[:, 0:64], pt1[0:P, 0:64])
            pt2 = psT.tile([P, P], f32, tag="tp", space="PSUM")
            nc.tensor.transpose(pt2[0:F - P, 0:64], srcT[0:64, P:F],
                                ident32[0:64, 0:64])
            nc.vector.tensor_copy(d1[0:F - P, 0:64], pt2[0:F - P, 0:64])
        h2_ps = psM.tile([64, F], f32, tag="m", space="PSUM")
        nc.tensor.matmul(h2_ps[:], a2T0[:, 0:64], wm2r[0][:],
                         start=True, stop=False)
        nc.tensor.matmul(h2_ps[:], a2T1[0:F - P, 0:64], wm2r[1][0:F - P, :],
                         start=False, stop=False)
        nc.tensor.matmul(h2_ps[:], m1T0[:, 0:64], wm2s[0][:],
                         start=False, stop=False)
        nc.tensor.matmul(h2_ps[:], m1T1[0:F - P, 0:64], wm2s[1][0:F - P, :],
                         start=False, stop=True)
        t_m2 = sb.tile([64, F], f32, tag="mol3")
        nc.vector.tensor_tensor(t_m2[:], h2_ps[:], bm2r[0:64, :], op=ADD)
        m2_sb = sb.tile([64, F], f32, tag="mol4", bufs=1)
        nc.scalar.activation(m2_sb[:], t_m2[:], RELU)

        wlm = (S32b("wlma"), S32b("wlmb"))
        blm = S32b("blm_col")
        mcol0 = sb.tile([P, 1], f32, tag="mc0", bufs=1)
        mcol1 = sb.tile([P, 1], f32, tag="mc1", bufs=1)
        pool_ps = psM.tile([P, 1], f32, tag="m", space="PSUM")
        nc.tensor.matmul(pool_ps[0:P, :], m2_sb[0:64, 0:P], ones32[0:64, :],
                         start=True, stop=True)
        nc.scalar.activation(mcol0[:], pool_ps[0:P, :], COPY, scale=1.0 / 64.0)
        pool_ps2 = psM.tile([P, 1], f32, tag="m", space="PSUM")
        nc.tensor.matmul(pool_ps2[0:F - P, :], m2_sb[0:64, P:F],
                         ones32[0:64, :], start=True, stop=True)
        nc.scalar.activation(mcol1[0:F - P, :], pool_ps2[0:F - P, :], COPY,
                             scale=1.0 / 64.0)
        mvec = sb.tile([P, 1], f32, tag="mvec", bufs=1)
        mm_ps = psM.tile([P, 1], f32, tag="m", space="PSUM")
        nc.tensor.matmul(mm_ps[:], wlm[0][:], mcol0[:], start=True, stop=False)
        nc.tensor.matmul(mm_ps[:], wlm[1][0:F - P, :], mcol1[0:F - P, :],
                         start=False, stop=True)
        nc.scalar.activation(mvec[:], mm_ps[:], RELU, bias=blm[:])

        # ---- layer 2 -> h3 -> AG3 ----
        layer_tail(agg_blocks(hload2(h_full[0])), wc3, bc_rep[1],
                   h_slice[1], ag_halves=h_full[1])
        w1ct = load(wp, "W1ct", eng=nc.gpsimd)     # [128, 12000] f16

        # head weight batch 2
        bl3c = SR("bl3c_row")
        wl3cpk = load(wp, "Wl3c_pk", eng=nc.sync)
        bcat1 = SR("bcat1_row")
        wcat1pk = load(wp, "Wcat1_pk", eng=nc.sync)
        wcat2pk = S32b("wcat2_pk")
        bcat2 = SR("bcat2_t")

        # ---- layer 3 -> z = c3 @ Wc4 [512, 3] -> AG-z ----
        layer_tail(agg_blocks(hload2(h_full[1])), wc4, bc_rep[2], z_slice,
                   ncols=3)
        allgather(z_slice[0:CLL_NPC, 0:3], z_full.ap())

        # ---- fusion head minis (fill the AG-z window) ----
        def mm_chain(p_rows, n_cols, k_steps, act_bias, out_tag):
            acc = sb.tile([p_rows, n_cols], f32, tag=out_tag + "a")
            for k in range(k_steps):
                lhsT, rhs = yield k
                pst = psM.tile([p_rows, n_cols], f32, tag="m", space="PSUM")
                for och in range(n_cols):
                    nc.tensor.matmul(pst[:, och:och + 1], lhsT(och), rhs,
                                     start=True, stop=True)
                if k == 0:
                    nc.vector.tensor_copy(acc[:], pst[:])
                else:
                    nc.vector.tensor_tensor(acc[:], acc[:], pst[:], op=ADD)
            o = sb.tile([p_rows, n_cols], f32, tag=out_tag, bufs=1)
            for och in range(n_cols):
                nc.scalar.activation(o[:, och:och + 1], acc[:, och:och + 1],
                                     RELU, bias=act_bias[:, och:och + 1])
            yield o

        def run_chain(p_rows, n_cols, pieces, act_bias, out_tag):
            gen = mm_chain(p_rows, n_cols, len(pieces), act_bias, out_tag)
            k = next(gen)
            while True:
                r = gen.send(pieces[k])
                if not isinstance(r, int):
                    return r
                k = r

        bvec = run_chain(P, 1, [
            (lambda o: wlb[0][:, 0:128], bgc0[:]),
            (lambda o: wlb[1][0:F - P, 0:128], bgc1[0:F - P, :]),
        ], blb, "bvec")

        d1 = run_chain(125, 4, [
            (lambda o: wd1[0][:, o * 125:(o + 1) * 125], mvec[:]),
            (lambda o: wd1[1][:, o * 125:(o + 1) * 125], bvec[:]),
        ], bd1, "d1")

        d2 = run_chain(P, 2, [
            (lambda o, k=k: wd2t[k][:, o * P:(o + 1) * P], d1[:, k:k + 1])
            for k in range(4)
        ], bd2, "d2")

        # ---- P^T = sum_s z_s^T Q_s, h4, dense-1 partials ----
        zf = sb.tile([P, CLL_NCH, 3], f16, tag="zf", bufs=1)
        nc.sync.dma_start(
            zf[:], z_full.ap()[0:CLL_NF, :].rearrange("(c p) f -> p c f", p=P))
        ptz = psM.tile([3, CLL_NPC], f32, tag="m", space="PSUM")
        for s in range(CLL_NCH):
            pi = piece_of(s)
            c0 = (s - PIECES[pi][0]) * CLL_NBLK * P
            nc.tensor.matmul(ptz[:], zf[:, s, 0:3],
                             qt_p[pi][:, c0:c0 + CLL_NBLK * P],
                             start=(s == 0), stop=(s == CLL_NCH - 1))
        ptz_sb = sb.tile([3, CLL_NPC], f32, tag="ptzsb", bufs=1)
        nc.vector.tensor_copy(ptz_sb[:], ptz[:])

        h4c = []
        for b in range(CLL_NBLK):
            pt4 = psT.tile([P, P], f32, tag="tp", space="PSUM")
            nc.tensor.transpose(pt4[0:P, 0:3],
                                ptz_sb[0:3, b * P:(b + 1) * P],
                                ident32[0:3, 0:3])
            th4 = sb.tile([P, 3], f32, tag="th4")
            nc.vector.tensor_tensor(th4[:], pt4[0:P, 0:3], bc4r[:, 0:3],
                                    op=ADD)
            hb = sb.tile([P, 3], f16, tag=f"h4c{b}", bufs=1)
            nc.scalar.activation(hb[:], th4[:], RELU)
            h4c.append(hb)

        # dense-1 partials, block-major so mms start as soon as h4c[0] lands
        JORDER = [ch * 4 + b for b in range(CLL_NBLK) for ch in range(3)]
        dsum = sb.tile([1, 1024], f32, tag="rowb", bufs=1)
        nc.vector.memset(dsum[0:1, 1000:1024], 0.0)
        for half in range(2):
            psd = psM.tile([1, 500], f32, tag="m", space="PSUM")
            for pos, j in enumerate(JORDER):
                ch, b = j // 4, j % 4
                nc.tensor.matmul(psd[:], h4c[b][:, ch:ch + 1],
                                 w1ct[:, j * 1000 + half * 500:
                                      j * 1000 + half * 500 + 500],
                                 start=(pos == 0), stop=(pos == 11))
            nc.vector.tensor_copy(dsum[0:1, half * 500:half * 500 + 500],
                                  psd[0:1, :])
        nc.gpsimd.dma_start(ar_in.ap()[0:1024, None], dsum[0:1, :])

        nc.gpsimd.collective_compute(
            "AllReduce", mybir.AluOpType.add, replica_groups=RG,
            ins=[ar_in.ap()], outs=[ar_out.ap()])

        # ---- fusion head (replicated) ----
        c1in = sb.tile([P, 8], f32, tag="c1in", bufs=1)
        nc.sync.dma_start(c1in[:], ar_out.ap().rearrange("(j p) -> p j", p=P))
        c1t = sb.tile([P, 8], f32, tag="c1t", bufs=1)
        nc.vector.tensor_tensor(c1t[:], c1in[:], bl1c[:], op=ADD)
        c1h = sb.tile([P, 8], f16, tag="c1h", bufs=1)
        nc.scalar.activation(c1h[:], c1t[:], RELU)

        def rowstage(lhs_cols, rhs_pk, rhs_cw, ncols, bias_row, tag,
                     out_f16=True):
            """out_row[1, ncols(+pad)] = relu(sum_j lhs_cols[j]^T rhs_j + b)."""
            npad = max(ncols, 1024) if ncols > 512 else ncols
            rb = sb.tile([1, npad], f32, tag="rowb", bufs=1)
            if npad > ncols:
                nc.vector.memset(rb[0:1, ncols:npad], 0.0)
            for h0 in range(0, ncols, 500):
                hw = min(500, ncols - h0)
                psr = psM.tile([1, hw], f32, tag="m", space="PSUM")
                for j, col in enumerate(lhs_cols):
                    nc.tensor.matmul(psr[:], col,
                                     rhs_pk[:, j * rhs_cw + h0:
                                            j * rhs_cw + h0 + hw],
                                     start=(j == 0),
                                     stop=(j == len(lhs_cols) - 1))
                nc.vector.tensor_tensor(rb[0:1, h0:h0 + hw], psr[0:1, :],
                                        bias_row[0:1, h0:h0 + hw], op=ADD)
            ro = sb.tile([1, npad], f16 if out_f16 else f32, tag=tag + "o",
                         bufs=1)
            nc.scalar.activation(ro[0:1, :], rb[0:1, :], RELU)
            return ro

        def rowcols(row, n, tag, idf):
            cols = sb.tile([P, n], row.dtype, tag=tag, bufs=1)
            for j in range(n):
                ptj = psT.tile([P, P], row.dtype, tag="tp", name="ptj",
                               space="PSUM")
                nc.tensor.transpose(ptj[0:P, 0:1], row[0:1, j * P:(j + 1) * P],
                                    idf[0:1, 0:1])
                nc.vector.tensor_copy(cols[:, j:j + 1], ptj[0:P, 0:1])
            return cols

        c1cols = [c1h[:, j:j + 1] for j in range(8)]
        c2h = rowstage(c1cols, wl2cpk, 1000, 1000, bl2c, "c2")
        c2c = rowcols(c2h, 8, "c2c", ident16)
        c3h = rowstage([c2c[:, j:j + 1] for j in range(8)], wl3cpk, 256, 256,
                       bl3c, "c3")
        d2h = sb.tile([P, 2], f16, tag="d2h", bufs=1)
        nc.vector.tensor_copy(d2h[:], d2[:])
        c3c = rowcols(c3h, 2, "c3c", ident16)
        ucols_in = [d2h[:, 0:1], d2h[:, 1:2], c3c[:, 0:1], c3c[:, 1:2]]
        uact = rowstage(ucols_in, wcat1pk, 1000, 1000, bcat1, "u",
                        out_f16=False)
        ucols = rowcols(uact, 8, "ucols", ident32)
        pso = psM.tile([1, 1], f32, tag="m", space="PSUM")
        for k in range(8):
            nc.tensor.matmul(pso[:], ucols[:, k:k + 1], wcat2pk[:, k:k + 1],
                             start=(k == 0), stop=(k == 7))
        osb = sb.tile([1, 1], f32, tag="osb", bufs=1)
        nc.scalar.activation(osb[:], pso[:], RELU, bias=bcat2[:])
        nc.sync.dma_start(out[0:1, 0:1], osb[:])


# ------------------------------------------------------------------- entry

_CACHE = {}


def kernel(**inputs):
    in_maps, meta = prep_inputs(inputs)
    if "nc" not in _CACHE:
        _CACHE["nc"] = build_program(meta)
    nc = _CACHE["nc"]
    res = run_bass_kernel_spmd(nc, in_maps, core_ids=list(range(NCORES)))
    return np.asarray(res.results[0]["out"], np.float32)


# revision 36
# speedup vs baseline: 1.1118x; 1.1118x over previous
"""Trainium2 Bass kernel for nn_DrugRank (GNN message passing), 8 NeuronCores.

Architecture (v3 — replicated L1 transform, z-trick tail):

  - Bio branch pruned to the 2-hop in-neighborhood of node 49999 (the only
    row the reference consumes); replicated per core.
  - cll graph (3451 nodes, 55216 edges, 4 GCN layers): dst-node sharded,
    512 nodes (4 blocks of 128) per core. GCN normalization folded host-side
    into dense per-(src-chunk, dst-block) adjacency tiles Q.
  - Layer 1: h1 = x_cll @ Wc1 is collective-free (x replicated input), so
    every core computes the FULL h1 (27 chunks) locally — no AllGather and
    the one-time CC barrier (~30-50us) overlaps this compute instead of
    idling the PE.
  - Layers 1-3 aggregate via 27x4 PSUM-accumulated 128x128x200 matmuls;
    per-layer AllGather of the 200KB transformed slice (AG2, AG3 only).
  - Layer 4 via associativity: (Q^T c3) @ Wc4 = Q^T (c3 @ Wc4). Each core
    computes z = c3_own @ Wc4 [512,3], AllGathers 3KB instead of 204KB,
    then P^T = sum_s z_s^T Q_s as 27 skinny matmuls. Saves AG4's wire time
    and its dead window.
  - mol branch, bio branch, and fusion-head mini-chains are emitted into
    the AG windows. Dense-1 row-sharded + AllReduce, head replicated.
"""

import numpy as np

import concourse.bacc as bacc
import concourse.bass as bass
import concourse.mybir as mybir
import concourse.tile as tile
from concourse.bass_utils import run_bass_kernel_spmd

NCORES = 8
P = 128
F = 200

CLL_N, CLL_E, CLL_PAD, CLL_NPC = 3451, 55216, 4096, 512
CLL_NBLK = CLL_NPC // P                 # 4 dst blocks / core
CLL_NCH = 27                            # src chunks with real nodes
CLL_NF = CLL_NCH * P                    # 3456 packed src nodes
N_BIO = 50000
BIO_S2, BIO_S1 = 768, 128               # padded bio 2-hop sets
BIO_NCH = BIO_S2 // P                   # 6
MOL_N, MOL_E = 64, 128

f32 = mybir.dt.float32
f16 = mybir.dt.float16
i16 = mybir.dt.int16
RELU = mybir.ActivationFunctionType.Relu
COPY = mybir.ActivationFunctionType.Copy
EQ = mybir.AluOpType.is_equal
MUL = mybir.AluOpType.mult
ADD = mybir.AluOpType.add

# Small weights are packed into a few blob tensors (one DMA each) because
# every dma_start costs ~0.6-1.3us of engine-queue time and the HWDGE
# completion-tracking window stalls the queue after ~10 outstanding DMAs.
# Each entry: (name, rows, cols); rows are zero-padded to the blob height.
BLOB16_SPEC = [
    ("wc2a", 128, F), ("wc2b", 72, F),
    ("wc3a", 128, F), ("wc3b", 72, F),
    ("wc4a", 128, 3), ("wc4b", 72, 3),
    ("wb2a", 128, F), ("wb2b", 72, F),
    ("qb1", 128, BIO_NCH * BIO_S1), ("qb2", 128, 1),
    ("xbT0", 128, BIO_S2), ("xbT1", 128, BIO_S2),
    ("wb1_0", 128, F), ("wb1_1", 128, F),
    ("ident16", 128, 128),
]
BLOB32E_SPEC = [
    ("bc1_rep", 128, F), ("bc2_rep", 128, F), ("bc3_rep", 128, F),
    ("bc4_rep", 128, 4),
    ("iota32", 128, 128), ("ident32", 128, 128), ("ones32", 128, 1),
    ("mol_slot", 128, 1),
]
BLOB32A_SPEC = [
    ("xmolT", 64, MOL_N),
    ("wm1r", 64, F), ("wm1s", 64, F),
    ("bm1_rep", 64, F), ("bm2_rep", 64, F),
]
BLOB32B_SPEC = [
    ("bb1_rep", 128, F),
    ("wm2ra", 128, F), ("wm2rb", 72, F),
    ("wm2sa", 128, F), ("wm2sb", 72, F),
    ("wlma", 128, 128), ("wlmb", 72, 128),
    ("wlba", 128, 128), ("wlbb", 72, 128),
    ("wd1_0", 128, 500), ("wd1_1", 128, 500),
    ("wd2_0", 125, 256), ("wd2_1", 125, 256),
    ("wd2_2", 125, 256), ("wd2_3", 125, 256),
    ("bd1_t", 125, 4), ("bd2_t", 128, 2),
    ("blm_col", 128, 1), ("blb_col", 128, 1),
    ("bl1c_pk", 128, 8), ("wcat2_pk", 128, 8),
]
BROW_SPEC = [
    ("bb2_row", 1, F), ("bl2c_row", 1, 1024), ("bl3c_row", 1, 256),
    ("bcat1_row", 1, 1024), ("bcat2_t", 1, 1),
]


def _blob_offsets(spec):
    offs, c = {}, 0
    for name, rows, cols in spec:
        offs[name] = (c, rows, cols)
        c += cols
    return offs, c


BLOB16_OFF, BLOB16_N = _blob_offsets(BLOB16_SPEC)
BLOB32E_OFF, BLOB32E_N = _blob_offsets(BLOB32E_SPEC)
BLOB32A_OFF, BLOB32A_N = _blob_offsets(BLOB32A_SPEC)
BLOB32B_OFF, BLOB32B_N = _blob_offsets(BLOB32B_SPEC)
BROW_OFF, BROW_N = _blob_offsets(BROW_SPEC)
_BLOBS = [("blob16", BLOB16_OFF), ("blob32e", BLOB32E_OFF),
          ("blob32a", BLOB32A_OFF), ("blob32b", BLOB32B_OFF),
          ("brow", BROW_OFF)]


def _pack_blob(spec, parts, height, dt):
    offs, total = _blob_offsets(spec)
    blob = np.zeros((height, total), dt)
    for name, rows, cols in spec:
        a = np.asarray(parts[name])
        assert a.shape == (rows, cols), (name, a.shape, rows, cols)
        blob[:rows, offs[name][0]:offs[name][0] + cols] = a
    return np.ascontiguousarray(blob)


def blob_get(m, name):
    """Extract an unpadded sub-array from the packed blobs (test helper)."""
    for key, offs in _BLOBS:
        if name in offs:
            c0, rows, cols = offs[name]
            return np.asarray(m[key])[0:rows, c0:c0 + cols]
    raise KeyError(name)


class Slab:
    """Column window of a blob tile, sliceable like a standalone tile."""

    def __init__(self, tile, off, rows, cols):
        self.t, self.off, self.rows, self.cols = tile, off, rows, cols

    def __getitem__(self, key):
        if not isinstance(key, tuple):
            key = (key, slice(None))
        rs, cs = key
        r0 = rs.start if rs.start is not None else 0
        r1 = rs.stop if rs.stop is not None else self.rows
        c0 = cs.start if cs.start is not None else 0
        c1 = cs.stop if cs.stop is not None else self.cols
        return self.t[r0:r1, self.off + c0:self.off + c1]


# ---------------------------------------------------------------- host prep

def _pack_idx16(flat):
    n = len(flat)
    a16 = np.asarray(flat, np.int16).reshape(n // 16, 16).T
    return np.ascontiguousarray(np.tile(a16, (8, 1)))


def _pack_slots(flat, dtype=np.float16):
    n = len(flat)
    return np.ascontiguousarray(
        np.asarray(flat, np.float64).astype(dtype).reshape(n // P, P).T)


def _col(v):
    return np.ascontiguousarray(np.asarray(v, np.float32).reshape(-1, 1))


def _rep(v, rows=P):
    return np.ascontiguousarray(
        np.tile(np.asarray(v, np.float32).reshape(1, -1), (rows, 1)))


def _btile(v, p, n):
    return np.ascontiguousarray(np.asarray(v, np.float32).reshape(n, p).T)


def _rowpad(v, n):
    """[m] -> [1, n] zero-padded row."""
    v = np.asarray(v, np.float32).reshape(-1)
    o = np.zeros((1, n), np.float32)
    o[0, :len(v)] = v
    return o


def _rowpack(w, rows_pad, cols, dt=np.float16):
    """[m, cols] -> [128, (rows_pad//128)*cols]: [p, j*cols+q] = w[j*128+p, q]."""
    w = np.asarray(w, np.float32)
    wp = np.zeros((rows_pad, cols), np.float32)
    wp[:w.shape[0]] = w
    nj = rows_pad // P
    return np.ascontiguousarray(
        wp.reshape(nj, P, cols).transpose(1, 0, 2).reshape(P, nj * cols)
    ).astype(dt)


def _cll_q(edge, dinv):
    """Dense normalized adjacency, [CLL_PAD, CLL_PAD] f32."""
    src = edge[0].astype(np.int64)
    dst = edge[1].astype(np.int64)
    q = np.zeros((CLL_PAD, CLL_PAD), np.float32)
    np.add.at(q, (src, dst), (dinv[src] * dinv[dst]).astype(np.float32))
    di = np.arange(CLL_N)
    q[di, di] += (dinv[:CLL_N] * dinv[:CLL_N]).astype(np.float32)
    return q


def _bio_prune(edge, x_bio):
    """2-hop in-neighborhood of node N_BIO-1 -> (xbT_sel, Qb1_pk, Qb2)."""
    src = edge[0].astype(np.int64)
    dst = edge[1].astype(np.int64)
    deg = np.bincount(dst, minlength=N_BIO).astype(np.float64) + 1.0
    dinv = 1.0 / np.sqrt(deg)
    tgt = N_BIO - 1

    m2 = dst == tgt
    s1 = np.unique(np.concatenate([src[m2], [tgt]]))
    assert len(s1) <= BIO_S1, len(s1)
    pos1 = np.full(N_BIO, -1, np.int64)
    pos1[s1] = np.arange(len(s1))

    m1 = pos1[dst] >= 0
    e1s, e1d = src[m1], dst[m1]
    s2 = np.unique(np.concatenate([e1s, s1]))
    assert len(s2) <= BIO_S2, len(s2)
    pos2 = np.full(N_BIO, -1, np.int64)
    pos2[s2] = np.arange(len(s2))

    q1 = np.zeros((BIO_S2, BIO_S1), np.float32)
    np.add.at(q1, (pos2[e1s], pos1[e1d]),
              (dinv[e1s] * dinv[e1d]).astype(np.float32))
    q1[pos2[s1], pos1[s1]] += (dinv[s1] * dinv[s1]).astype(np.float32)

    q2 = np.zeros((BIO_S1, 1), np.float32)
    np.add.at(q2, (pos1[src[m2]], 0),
              (dinv[src[m2]] * dinv[tgt]).astype(np.float32))
    q2[pos1[tgt], 0] += np.float32(dinv[tgt] * dinv[tgt])

    xsel = np.zeros((BIO_S2, 256), np.float32)
    xsel[:len(s2)] = x_bio[s2]
    xbT = np.ascontiguousarray(xsel.T).astype(np.float16)      # [256, 768]
    # Qb1 packed [128, 6*128]: [p, s*128+d] = q1[s*128+p, d]
    q1pk = np.ascontiguousarray(
        q1.reshape(BIO_NCH, P, BIO_S1).transpose(1, 0, 2)
        .reshape(P, BIO_NCH * BIO_S1)).astype(np.float16)
    return xbT, q1pk, q2.astype(np.float16)


def prep_inputs(inp):
    meta = {}
    # ---- cll Q tiles ----
    dst = inp["edge_cll"][1].astype(np.int64)
    deg = np.bincount(dst, minlength=CLL_N).astype(np.float64) + 1.0
    dinv = np.zeros(CLL_PAD, np.float64)
    dinv[:CLL_N] = 1.0 / np.sqrt(deg)
    q = _cll_q(inp["edge_cll"], dinv)

    xcT = np.zeros((512, CLL_NF), np.float32)
    xcT[:, :CLL_N] = inp["x_cll"].T
    # full-graph xcllT packed [128, 4*3456]: [p, k*3456+n] = x_cll.T[k*128+p, n]
    xcllT_full = np.ascontiguousarray(
        xcT.reshape(4, P, CLL_NF).transpose(1, 0, 2)
        .reshape(P, 4 * CLL_NF)).astype(np.float16)

    # W1c regrouped: rows (node*3+ch) -> per core [128, 12*1000] f16,
    # col-block j = ch*4+blk, rows = local node p of that block.
    w1c = np.asarray(inp["Wl1c"], np.float32)                  # [10353, 1000]
    w1c_n = np.zeros((CLL_PAD, 3, 1000), np.float32)
    w1c_n[:CLL_N] = w1c.reshape(CLL_N, 3, 1000)

    xbT_sel, q1pk, q2 = _bio_prune(inp["edge_bio"], np.asarray(inp["x_bio"]))

    mol_s = inp["edge_mol"][0].astype(np.int64)
    mol_d = inp["edge_mol"][1].astype(np.int64)
    order = np.argsort(mol_d, kind="stable")
    mol_idx = _pack_idx16(mol_s[order])
    mol_slot = _pack_slots(mol_d[order].astype(np.float64), np.float32)

    iota = np.tile(np.arange(P, dtype=np.float32), (P, 1))
    ident = np.eye(P, dtype=np.float32)

    wc1 = np.asarray(inp["Wc1"], np.float32)                   # [512, 200]

    def _f16(x):
        return np.asarray(x, np.float16)

    def _f32(x):
        return np.asarray(x, np.float32)

    wb2 = _f16(inp["Wb2"])
    wm2r, wm2s = _f32(inp["Wm2r"]), _f32(inp["Wm2s"])
    wlm, wlb = _f32(inp["Wlm"]), _f32(inp["Wlb"])
    wd1, wd2 = _f32(inp["Wd1"]), _f32(inp["Wd2"])
    p16 = {
        "wc2a": _f16(inp["Wc2"])[:128], "wc2b": _f16(inp["Wc2"])[128:],
        "wc3a": _f16(inp["Wc3"])[:128], "wc3b": _f16(inp["Wc3"])[128:],
        "wc4a": _f16(inp["Wc4"])[:128], "wc4b": _f16(inp["Wc4"])[128:],
        "wb2a": wb2[:128], "wb2b": wb2[128:],
        "qb1": q1pk, "qb2": np.tile(q2, (1, 1)),
        "xbT0": xbT_sel[:128], "xbT1": xbT_sel[128:],
        "wb1_0": _f16(inp["Wb1"])[:128], "wb1_1": _f16(inp["Wb1"])[128:],
        "ident16": ident.astype(np.float16),
    }
    p32e = {
        "bc1_rep": _rep(inp["bc1"]), "bc2_rep": _rep(inp["bc2"]),
        "bc3_rep": _rep(inp["bc3"]),
        "bc4_rep": np.pad(_rep(inp["bc4"]), ((0, 0), (0, 1))),
        "iota32": iota, "ident32": ident,
        "ones32": np.ones((P, 1), np.float32),
        "mol_slot": mol_slot,
    }
    p32a = {
        "xmolT": np.ascontiguousarray(inp["x_mol"].T.astype(np.float32)),
        "wm1r": _f32(inp["Wm1r"]), "wm1s": _f32(inp["Wm1s"]),
        "bm1_rep": _rep(inp["bm1"], 64), "bm2_rep": _rep(inp["bm2"], 64),
    }
    p32b = {
        "bb1_rep": _rep(inp["bb1"]),
        "wm2ra": wm2r[:128], "wm2rb": wm2r[128:],
        "wm2sa": wm2s[:128], "wm2sb": wm2s[128:],
        "wlma": wlm[:128], "wlmb": wlm[128:],
        "wlba": wlb[:128], "wlbb": wlb[128:],
        "wd1_0": wd1[:128], "wd1_1": wd1[128:],
        "wd2_0": wd2[0:125], "wd2_1": wd2[125:250],
        "wd2_2": wd2[250:375], "wd2_3": wd2[375:500],
        "bd1_t": _btile(inp["bd1"], 125, 4),
        "bd2_t": _btile(inp["bd2"], 128, 2),
        "blm_col": _col(inp["blm"]), "blb_col": _col(inp["blb"]),
        "bl1c_pk": np.ascontiguousarray(
            _rowpad(inp["bl1c"], 1024).reshape(8, P).T),
        "wcat2_pk": _rowpack(inp["Wcat2"], 1024, 1, np.float32),
    }
    prow = {
        "bb2_row": np.ascontiguousarray(
            np.asarray(inp["bb2"], np.float32).reshape(1, -1)),
        "bl2c_row": _rowpad(inp["bl2c"], 1024),
        "bl3c_row": _rowpad(inp["bl3c"], 256),
        "bcat1_row": _rowpad(inp["bcat1"], 1024),
        "bcat2_t": np.asarray(inp["bcat2"], np.float32).reshape(1, 1),
    }
    shared = {
        "xcllT": xcllT_full,
        "Wc1": np.ascontiguousarray(
            wc1.reshape(4, P, F).transpose(1, 0, 2)
            .reshape(P, 4 * F)).astype(np.float16),
        "x_mol": np.asarray(inp["x_mol"], np.float32),
        "mol_idx": mol_idx,
        "Wcat1_pk": _rowpack(inp["Wcat1"], 512, 1000),
        "Wl2c_pk": _rowpack(inp["Wl2c"], 1024, 1000),
        "Wl3c_pk": _rowpack(inp["Wl3c"], 1024, 256),
        "blob16": _pack_blob(BLOB16_SPEC, p16, P, np.float16),
        "blob32e": _pack_blob(BLOB32E_SPEC, p32e, P, np.float32),
        "blob32a": _pack_blob(BLOB32A_SPEC, p32a, P, np.float32),
        "blob32b": _pack_blob(BLOB32B_SPEC, p32b, P, np.float32),
        "brow": _pack_blob(BROW_SPEC, prow, 1, np.float32),
    }
    in_maps = []
    for c in range(NCORES):
        m = dict(shared)
        lo = c * CLL_NPC
        # Qt packed [128, 27*4*128]: [p, (s*4+b)*128+d] = q[s*128+p, lo+b*128+d]
        qc = q[:CLL_NF, lo:lo + CLL_NPC]
        m["Qt"] = np.ascontiguousarray(
            qc.reshape(CLL_NCH, P, CLL_NBLK, P).transpose(1, 0, 2, 3)
            .reshape(P, CLL_NCH * CLL_NBLK * P)).astype(np.float16)
        # W1ct [128, 12*1000]: [p, (ch*4+blk)*1000+q] = w1c_n[lo+blk*128+p, ch, q]
        wslice = w1c_n[lo:lo + CLL_NPC]                         # [512, 3, 1000]
        m["W1ct"] = np.ascontiguousarray(
            wslice.reshape(CLL_NBLK, P, 3, 1000).transpose(1, 2, 0, 3)
            .reshape(P, 12 * 1000)).astype(np.float16)
        in_maps.append(m)
    return in_maps, meta


# ------------------------------------------------------------ device program

RG = [list(range(NCORES))]


def _declare_inputs(nc):
    spec = {
        "xcllT": ([P, 4 * CLL_NF], f16),
        "Qt": ([P, CLL_NCH * CLL_NBLK * P], f16),
        "Wc1": ([P, 4 * F], f16),
        "W1ct": ([P, 12 * 1000], f16),
        "x_mol": ([MOL_N, 64], f32),
        "mol_idx": ([P, 8], i16),
        "Wcat1_pk": ([P, 4 * 1000], f16),
        "Wl2c_pk": ([P, 8 * 1000], f16),
        "Wl3c_pk": ([P, 8 * 256], f16),
        "blob16": ([P, BLOB16_N], f16),
        "blob32e": ([P, BLOB32E_N], f32),
        "blob32a": ([P, BLOB32A_N], f32),
        "blob32b": ([P, BLOB32B_N], f32),
        "brow": ([1, BROW_N], f32),
    }
    return {k: nc.dram_tensor(k, s, d, kind="ExternalInput")
            for k, (s, d) in spec.items()}


def build_program(meta=None, repeat=1):
    nc = bacc.Bacc("TRN2", target_bir_lowering=False, debug=False,
                   enable_asserts=False, num_devices=NCORES,
                   num_swdge_queues=4)
    io = _declare_inputs(nc)
    out = nc.dram_tensor("out", [1, 1], f32, kind="ExternalOutput")

    # h_slice/h_full for layers 2 and 3 (AG2, AG3, each split in 2 halves);
    # z for the layer-4 trick
    h_slice = [nc.dram_tensor(f"h{l}_slice", [CLL_NPC, F], f16,
                              kind="Internal") for l in range(2)]
    h_full = [[nc.dram_tensor(f"h{l}_full{h}", [CLL_PAD // 2, F], f16,
                              kind="Internal", addr_space="Shared")
               for h in range(2)] for l in range(2)]
    z_slice = nc.dram_tensor("z_slice", [CLL_NPC, 3], f16, kind="Internal")
    z_full = nc.dram_tensor("z_full", [CLL_PAD, 3], f16, kind="Internal",
                            addr_space="Shared")
    bar_in = nc.dram_tensor("bar_in", [8], f32, kind="Internal")
    bar_out = nc.dram_tensor("bar_out", [8], f32, kind="Internal",
                             addr_space="Shared")
    m1_dram = nc.dram_tensor("m1_dram", [MOL_N, 256], f32, kind="Internal")
    ar_in = nc.dram_tensor("ar_in", [1024], f32, kind="Internal")
    ar_out = nc.dram_tensor("ar_out", [1024], f32, kind="Internal",
                            addr_space="Shared")

    with tile.TileContext(nc) as tc:
        for _ in range(repeat):
            _build(nc, tc, io, out, h_slice, h_full, z_slice, z_full,
                   m1_dram, ar_in, ar_out, bar_in, bar_out)
    nc.compile()
    return nc


def _build(nc, tc, io, out, h_slice, h_full, z_slice, z_full,
           m1_dram, ar_in, ar_out, bar_in, bar_out):
    with (
        tc.tile_pool(name="const", bufs=1) as cp,
        tc.tile_pool(name="wp", bufs=1) as wp,
        tc.tile_pool(name="hp", bufs=2) as hp,
        tc.tile_pool(name="sb", bufs=2) as sb,
        tc.tile_pool(name="ct", bufs=1) as ctp,
        tc.tile_pool(name="psA", bufs=1, space="PSUM") as psA,
        tc.tile_pool(name="psT", bufs=2, space="PSUM") as psT,
        tc.tile_pool(name="psM", bufs=2, space="PSUM") as psM,
    ):
        def load(pool, name, rows=None, cols=None, tag=None, dt=None,
                 eng=None):
            src = io[name]
            r = rows if rows is not None else src.shape[0]
            c = cols if cols is not None else src.shape[1]
            t = pool.tile([r, c], dt or src.dtype, tag=tag or name)
            (eng or nc.sync).dma_start(t[:], src[0:r, 0:c])
            return t

        def load2(name, tag, rows=F, cols=F, eng=None):
            """[rows>128, cols] -> two tiles [128, cols] + [rows-128, cols]."""
            a = load(wp, name, rows=P, cols=cols, tag=tag + "a", eng=eng)
            b = wp.tile([P, cols], io[name].dtype, tag=tag + "b")
            (eng or nc.sync).dma_start(b[0:rows - P, :], io[name][P:rows, 0:cols])
            return a, b

        def loadrows(name, nparts, cols, tag, rows=P, eng=None):
            """Tall [nparts*rows?, cols] tensor -> list of [128, cols] tiles."""
            ts = []
            for k in range(nparts):
                t = wp.tile([rows, cols], io[name].dtype, tag=f"{tag}{k}")
                (eng or nc.sync).dma_start(
                    t[:], io[name][k * rows:(k + 1) * rows, 0:cols])
                ts.append(t)
            return ts

        # ---- phase A inputs first: Wc1 + xcllT (8 half-chunk tiles over
        # the two HWDGE queues, first halves first), Qt pieces on SWDGE ----
        nc.gpsimd.collective_compute(
            "AllReduce", mybir.AluOpType.add, replica_groups=RG,
            ins=[bar_in.ap()], outs=[bar_out.ap()])
        wc1 = load(wp, "Wc1", eng=nc.scalar)       # [128, 4*200] f16, k-major
        b32e = load(wp, "blob32e", eng=nc.scalar)  # early-critical constants

        PIECES = [(0, 7), (7, 14), (14, 21), (21, CLL_NCH)]

        # xcllT tiled per (k, piece) and Qt per piece, all loads round-robin
        # over the 3 DMA queues in CONSUMPTION order, so the per-queue
        # ~100GB/s arrival front-runs the fused transform+aggregate loop.
        ENGS = [nc.sync, nc.scalar, nc.gpsimd]
        xckp = [[wp.tile([P, (s1 - s0) * P], f16, tag=f"xc{k}_{pi}",
                         name=f"xc{k}_{pi}")
                 for pi, (s0, s1) in enumerate(PIECES)] for k in range(4)]
        qt_p = [wp.tile([P, (s1 - s0) * CLL_NBLK * P], f16, tag=f"Qt{pi}",
                        name=f"Qt{pi}")
                for pi, (s0, s1) in enumerate(PIECES)]
        ei = 0
        for pi, (s0, s1) in enumerate(PIECES):
            for k in range(4):
                ENGS[ei % 3].dma_start(
                    xckp[k][pi][:],
                    io["xcllT"][:, k * CLL_NF + s0 * P:k * CLL_NF + s1 * P])
                ei += 1
            ENGS[ei % 3].dma_start(
                qt_p[pi][:],
                io["Qt"][:, s0 * CLL_NBLK * P:s1 * CLL_NBLK * P])
            ei += 1

        def piece_of(s):
            return next(i for i, (a, b) in enumerate(PIECES) if a <= s < b)

        def qtcol(s, b):
            pi = piece_of(s)
            c0 = ((s - PIECES[pi][0]) * CLL_NBLK + b) * P
            return qt_p[pi][:, c0:c0 + P]

        # ---- phase A fused with L1 aggregation: per piece, transform
        # h1 chunks then immediately aggregate them, so the agg matmuls of
        # piece p hide the xcllT DMA tail for piece p+1 ----
        h1p = []
        for pi, (s0, s1) in enumerate(PIECES):
            h1p.append(hp.tile([P, s1 - s0, F], f16, tag=f"hft{pi}",
                               name=f"hft{pi}"))
        h1pss = [psA.tile([P, F], f32, tag=f"agg{b}", name=f"agg{b}",
                          space="PSUM") for b in range(CLL_NBLK)]
        for pi, (s0, s1) in enumerate(PIECES):
            for s in range(s0, s1):
                ps = psM.tile([P, F], f32, tag="m", space="PSUM")
                for k in range(4):
                    nc.tensor.matmul(
                        ps[:], xckp[k][pi][:, (s - s0) * P:(s - s0 + 1) * P],
                        wc1[:, k * F:(k + 1) * F],
                        start=(k == 0), stop=(k == 3))
                nc.vector.tensor_copy(h1p[pi][:, s - s0, :], ps[:])
            for s in range(s0, s1):
                for b in range(CLL_NBLK):
                    nc.tensor.matmul(h1pss[b][:], qtcol(s, b),
                                     h1p[pi][:, s - s0, 0:F],
                                     start=(s == 0), stop=(s == CLL_NCH - 1))

        # blob loads: one DMA each (trigger cost + the HWDGE completion
        # window make many small DMAs poisonous), then early mol prep
        mol_idx_sb = load(cp, "mol_idx", eng=nc.scalar)
        b16 = load(wp, "blob16", eng=nc.scalar)
        browt = load(wp, "brow", eng=nc.scalar)
        b32a = load(wp, "blob32a", eng=nc.sync)
        b32b = load(wp, "blob32b", eng=nc.sync)

        def _slab(tile, offs, name):
            c0, r, c = offs[name]
            return Slab(tile, c0, r, c)

        def S16(n):
            return _slab(b16, BLOB16_OFF, n)

        def S32a(n):
            return _slab(b32a, BLOB32A_OFF, n)

        def S32e(n):
            return _slab(b32e, BLOB32E_OFF, n)

        def S32b(n):
            return _slab(b32b, BLOB32B_OFF, n)

        def SR(n):
            return _slab(browt, BROW_OFF, n)

        iota32 = S32e("iota32")
        ident32 = S32e("ident32")
        ident16 = S16("ident16")
        ones32 = S32e("ones32")
        mol_slot_sb = S32e("mol_slot")
        xmolT_sb = S32a("xmolT")
        v1 = sb.tile([P, 1, 64], f32, tag="vm")
        nc.gpsimd.dma_gather(v1[:], io["x_mol"].ap(), mol_idx_sb[:],
                             MOL_E, MOL_E, 64)

        wc2 = (S16("wc2a"), S16("wc2b"))
        bc_rep = [S32e("bc1_rep"), S32e("bc2_rep"), S32e("bc3_rep")]

        def allgather(src, dst):
            nc.gpsimd.collective_compute(
                "AllGather", mybir.AluOpType.bypass, replica_groups=RG,
                ins=[src], outs=[dst])

        def hload2(hAB):
            """Split-AG halves hA/hB [2048, F] -> chunk_srcs list of
            (tile, idx, s) in processing order (half A chunks first)."""
            srcs = []
            for half, hx in enumerate(hAB):
                src = hx.ap().rearrange("(c p) f -> p c f", p=P)
                for sub in range(2):
                    t = hp.tile([P, 8, F], f16, tag=f"hft{half * 2 + sub}",
                                name=f"hft{half * 2 + sub}")
                    nc.sync.dma_start(t[:], src[:, sub * 8:(sub + 1) * 8, :])
                    for c in range(sub * 4, sub * 4 + 4):
                        for b in range(2):
                            s = 4 * c + 2 * half + b
                            if s < CLL_NCH:
                                srcs.append((t, (c - sub * 4) * 2 + b, s))
            return srcs

        def transpose_to(src_sb, dst0, dst1, bcol):
            """src [128, 200] f32 -> dst0[128, bcol:+128], dst1[72, bcol:+128] f16."""
            pt = psT.tile([P, P], f32, tag="tp", space="PSUM")
            nc.tensor.transpose(pt[0:P, 0:P], src_sb[:, 0:P], ident32[:])
            nc.vector.tensor_copy(dst0[:, bcol:bcol + P], pt[0:P, 0:P])
            pt2 = psT.tile([P, P], f32, tag="tp", space="PSUM")
            nc.tensor.transpose(pt2[0:F - P, 0:P], src_sb[:, P:F], ident32[:])
            nc.vector.tensor_copy(dst1[0:F - P, bcol:bcol + P],
                                  pt2[0:F - P, 0:P])

        def agg_blocks(chunk_srcs):
            """4 PSUM accumulators over an arbitrary chunk processing order
            (starts as soon as the first source tile is available)."""
            pss = [psA.tile([P, F], f32, tag=f"agg{b}", name=f"agg{b}",
                            space="PSUM") for b in range(CLL_NBLK)]
            n = len(chunk_srcs)
            for pos, (t, idx, s) in enumerate(chunk_srcs):
                for b in range(CLL_NBLK):
                    nc.tensor.matmul(pss[b][:], qtcol(s, b),
                                     t[:, idx, 0:F],
                                     start=(pos == 0), stop=(pos == n - 1))
            return pss

        def layer_tail(pss, wnext, brep, dst_dram, ncols=F, ag_halves=None):
            """relu(+bias), transform by wnext ([128,c]+[72,c] tiles), store
            [512, ncols] f16 slices; optionally trigger the half-AllGathers
            after blocks 1 and 3."""
            cT0 = ctp.tile([P, CLL_NPC], f16, tag="cT0")
            cT1 = ctp.tile([P, CLL_NPC], f16, tag="cT1")
            for b in range(CLL_NBLK):
                t2 = sb.tile([P, F], f32, tag="ev1")
                nc.vector.tensor_tensor(t2[:], pss[b][:], brep[:], op=ADD)
                cblk = sb.tile([P, F], f32, tag="cblk", bufs=3)
                nc.scalar.activation(cblk[:], t2[:], RELU)
                transpose_to(cblk, cT0, cT1, b * P)
                wa, wb_ = wnext
                ph = psM.tile([P, ncols], f32, tag="m", space="PSUM")
                nc.tensor.matmul(ph[:], cT0[:, b * P:(b + 1) * P],
                                 wa[:, 0:ncols], start=True, stop=False)
                nc.tensor.matmul(ph[:], cT1[0:F - P, b * P:(b + 1) * P],
                                 wb_[0:F - P, 0:ncols], start=False, stop=True)
                hst = sb.tile([P, ncols], f16, tag="hst", bufs=3)
                nc.vector.tensor_copy(hst[:], ph[:])
                nc.sync.dma_start(dst_dram[b * P:(b + 1) * P, 0:ncols],
                                  hst[:])
                if ag_halves is not None and b % 2 == 1:
                    half = b // 2
                    allgather(dst_dram[half * 2 * P:(half * 2 + 2) * P,
                                       0:ncols],
                              ag_halves[half].ap())

        # ---- layer 1 tail -> h2 -> AG2 ----
        layer_tail(h1pss, wc2, bc_rep[0], h_slice[0], ag_halves=h_full[0])

        # ---- bio mini-branch (fills the AG2/barrier window) ----
        xbT = [S16("xbT0"), S16("xbT1")]
        qb1 = S16("qb1")
        qb2 = S16("qb2")
        wb1 = [S16("wb1_0"), S16("wb1_1")]
        wb2 = (S16("wb2a"), S16("wb2b"))
        bb1r = S32b("bb1_rep")
        bb2row = SR("bb2_row")
        wc3 = (S16("wc3a"), S16("wc3b"))
        wc4 = (S16("wc4a"), S16("wc4b"))
        bc4r = S32e("bc4_rep")
        h1b = sb.tile([P, BIO_NCH, F], f16, tag="h1b", bufs=1)
        for j in range(BIO_NCH):
            ps = psM.tile([P, F], f32, tag="m", space="PSUM")
            for k in range(2):
                nc.tensor.matmul(ps[:], xbT[k][:, j * P:(j + 1) * P],
                                 wb1[k][:],
                                 start=(k == 0), stop=(k == 1))
            nc.vector.tensor_copy(h1b[:, j, :], ps[:])
        psb = psM.tile([P, F], f32, tag="m", space="PSUM")
        for j in range(BIO_NCH):
            nc.tensor.matmul(psb[:], qb1[:, j * P:(j + 1) * P], h1b[:, j, 0:F],
                             start=(j == 0), stop=(j == BIO_NCH - 1))
        tb1 = sb.tile([P, F], f32, tag="ev1")
        nc.vector.tensor_tensor(tb1[:], psb[:], bb1r[:], op=ADD)
        c1b = sb.tile([P, F], f32, tag="c1b", bufs=1)
        nc.scalar.activation(c1b[:], tb1[:], RELU)
        c1bT0 = sb.tile([P, P], f16, tag="c1bT0", bufs=1)
        c1bT1 = sb.tile([P, P], f16, tag="c1bT1", bufs=1)
        ptb = psT.tile([P, P], f32, tag="tp", space="PSUM")
        nc.tensor.transpose(ptb[0:P, 0:P], c1b[:, 0:P], ident32[:])
        nc.vector.tensor_copy(c1bT0[:], ptb[0:P, 0:P])
        ptb2 = psT.tile([P, P], f32, tag="tp", space="PSUM")
        nc.tensor.transpose(ptb2[0:F - P, 0:P], c1b[:, P:F], ident32[:])
        nc.vector.tensor_copy(c1bT1[0:F - P, :], ptb2[0:F - P, 0:P])
        ph2 = psM.tile([P, F], f32, tag="m", space="PSUM")
        nc.tensor.matmul(ph2[:], c1bT0[:, 0:P], wb2[0][:],
                         start=True, stop=False)
        nc.tensor.matmul(ph2[:], c1bT1[0:F - P, 0:P], wb2[1][0:F - P, :],
                         start=False, stop=True)
        h2b = sb.tile([P, F], f16, tag="h2b", bufs=1)
        nc.vector.tensor_copy(h2b[:], ph2[:])
        pr = psM.tile([1, F], f32, tag="m", space="PSUM")
        nc.tensor.matmul(pr[:], qb2[:], h2b[:], start=True, stop=True)
        tb2 = sb.tile([1, F], f32, tag="ev1")
        nc.vector.tensor_tensor(tb2[0:1, :], pr[0:1, :], bb2row[0:1, :], op=ADD)
        brow = sb.tile([1, F], f32, tag="brow", bufs=1)
        nc.scalar.activation(brow[0:1, :], tb2[0:1, :], RELU)
        # bvec column [200, 1] for the head
        bgc0 = sb.tile([P, 1], f32, tag="bgc0", bufs=1)
        bgc1 = sb.tile([P, 1], f32, tag="bgc1", bufs=1)
        prc = psT.tile([P, P], f32, tag="tp", space="PSUM")
        nc.tensor.transpose(prc[0:P, 0:1], brow[0:1, 0:P], ident32[0:1, 0:1])
        nc.vector.tensor_copy(bgc0[:], prc[0:P, 0:1])
        prc2 = psT.tile([P, P], f32, tag="tp", space="PSUM")
        nc.tensor.transpose(prc2[0:F - P, 0:1], brow[0:1, P:F],
                            ident32[0:1, 0:1])
        nc.vector.tensor_copy(bgc1[0:F - P, :], prc2[0:F - P, 0:1])

        # head weights (slabs) + big packed tables
        wlb = (S32b("wlba"), S32b("wlbb"))
        blb = S32b("blb_col")
        wd1 = (S32b("wd1_0"), S32b("wd1_1"))
        bd1 = S32b("bd1_t")
        wd2t = [S32b(f"wd2_{k}") for k in range(4)]
        bd2 = S32b("bd2_t")
        bl1c = S32b("bl1c_pk")
        bl2c = SR("bl2c_row")
        wl2cpk = load(wp, "Wl2c_pk", eng=nc.sync)

        # ---- mol branch part 1 + gather-2 trigger (SWDGE latency hides
        # under the AG2 window) ----
        wm1r = S32a("wm1r")
        wm1s = S32a("wm1s")
        bm1r = S32a("bm1_rep")
        bm2r = S32a("bm2_rep")
        mM = sb.tile([P, 64], f32, tag="Mmol", bufs=1)
        nc.vector.tensor_scalar(mM[:], iota32[:, 0:64], mol_slot_sb[:, 0:1],
                                None, op0=EQ)
        agg_ps = psM.tile([64, 64], f32, tag="m", space="PSUM")
        nc.tensor.matmul(agg_ps[:], mM[:], v1[:, 0, :], start=True, stop=True)
        agg_sb = sb.tile([64, 64], f32, tag="mol1")
        nc.vector.tensor_copy(agg_sb[:], agg_ps[:])
        pt = psT.tile([P, P], f32, tag="tp", space="PSUM")
        nc.tensor.transpose(pt[0:64, 0:64], agg_sb[0:64, 0:64],
                            ident32[0:64, 0:64])
        aggT = sb.tile([64, 64], f32, tag="mol2")
        nc.vector.tensor_copy(aggT[:], pt[0:64, 0:64])
        h1_ps = psM.tile([64, F], f32, tag="m", space="PSUM")
        nc.tensor.matmul(h1_ps[:], aggT[:], wm1r[:], start=True, stop=False)
        nc.tensor.matmul(h1_ps[:], xmolT_sb[:], wm1s[:], start=False, stop=True)
        t_m1 = sb.tile([64, F], f32, tag="mol3")
        nc.vector.tensor_tensor(t_m1[:], h1_ps[:], bm1r[0:64, :], op=ADD)
        m1_sb = sb.tile([64, F], f32, tag="mol4", bufs=1)
        nc.scalar.activation(m1_sb[:], t_m1[:], RELU)
        nc.gpsimd.dma_start(m1_dram[0:64, 0:F], m1_sb[:])

        wm2r = (S32b("wm2ra"), S32b("wm2rb"))
        wm2s = (S32b("wm2sa"), S32b("wm2sb"))
        v2 = sb.tile([P, 1, 256], f32, tag="vm2")
        nc.gpsimd.dma_gather(v2[:], m1_dram.ap(), mol_idx_sb[:],
                             MOL_E, MOL_E, 256)

        # ---- mol branch part 2 (fills the AG2 window) ----
        agg2_ps = psM.tile([64, F], f32, tag="m", space="PSUM")
        nc.tensor.matmul(agg2_ps[:], mM[:], v2[:, 0, 0:F], start=True, stop=True)
        agg2_sb = sb.tile([64, F], f32, tag="mol1")
        nc.vector.tensor_copy(agg2_sb[:], agg2_ps[:])
        a2T0 = sb.tile([P, 64], f32, tag="mol5")
        a2T1 = sb.tile([P, 64], f32, tag="mol6")
        m1T0 = sb.tile([P, 64], f32, tag="mol7")
        m1T1 = sb.tile([P, 64], f32, tag="mol8")
        for srcT, d0, d1 in ((agg2_sb, a2T0, a2T1), (m1_sb, m1T0, m1T1)):
            pt1 = psT.tile([P, P], f32, tag="tp", space="PSUM")
            nc.tensor.transpose(pt1[0:P, 0:64], srcT[0:64, 0:P],
                                ident32[0:64, 0:64])
            nc.vector.tensor_copy(d0[:, 0:64], pt1[0:P, 0:64])
            pt2 = psT.tile([P, P], f32, tag="tp", space="PSUM")
            nc.tensor.transpose(pt2[0:F - P, 0:64], srcT[0:64, P:F],
                                ident32[0:64, 0:64])
            nc.vector.tensor_copy(d1[0:F - P, 0:64], pt2[0:F - P, 0:64])
        h2_ps = psM.tile([64, F], f32, tag="m", space="PSUM")
        nc.tensor.matmul(h2_ps[:], a2T0[:, 0:64], wm2r[0][:],
                         start=True, stop=False)
        nc.tensor.matmul(h2_ps[:], a2T1[0:F - P, 0:64], wm2r[1][0:F - P, :],
                         start=False, stop=False)
        nc.tensor.matmul(h2_ps[:], m1T0[:, 0:64], wm2s[0][:],
                         start=False, stop=False)
        nc.tensor.matmul(h2_ps[:], m1T1[0:F - P, 0:64], wm2s[1][0:F - P, :],
                         start=False, stop=True)
        t_m2 = sb.tile([64, F], f32, tag="mol3")
        nc.vector.tensor_tensor(t_m2[:], h2_ps[:], bm2r[0:64, :], op=ADD)
        m2_sb = sb.tile([64, F], f32, tag="mol4", bufs=1)
        nc.scalar.activation(m2_sb[:], t_m2[:], RELU)

        wlm = (S32b("wlma"), S32b("wlmb"))
        blm = S32b("blm_col")
        mcol0 = sb.tile([P, 1], f32, tag="mc0", bufs=1)
        mcol1 = sb.tile([P, 1], f32, tag="mc1", bufs=1)
        pool_ps = psM.tile([P, 1], f32, tag="m", space="PSUM")
        nc.tensor.matmul(pool_ps[0:P, :], m2_sb[0:64, 0:P], ones32[0:64, :],
                         start=True, stop=True)
        nc.scalar.activation(mcol0[:], pool_ps[0:P, :], COPY, scale=1.0 / 64.0)
        pool_ps2 = psM.tile([P, 1], f32, tag="m", space="PSUM")
        nc.tensor.matmul(pool_ps2[0:F - P, :], m2_sb[0:64, P:F],
                         ones32[0:64, :], start=True, stop=True)
        nc.scalar.activation(mcol1[0:F - P, :], pool_ps2[0:F - P, :], COPY,
                             scale=1.0 / 64.0)
        mvec = sb.tile([P, 1], f32, tag="mvec", bufs=1)
        mm_ps = psM.tile([P, 1], f32, tag="m", space="PSUM")
        nc.tensor.matmul(mm_ps[:], wlm[0][:], mcol0[:], start=True, stop=False)
        nc.tensor.matmul(mm_ps[:], wlm[1][0:F - P, :], mcol1[0:F - P, :],
                         start=False, stop=True)
        nc.scalar.activation(mvec[:], mm_ps[:], RELU, bias=blm[:])

        # ---- layer 2 -> h3 -> AG3 ----
        layer_tail(agg_blocks(hload2(h_full[0])), wc3, bc_rep[1],
                   h_slice[1], ag_halves=h_full[1])
        w1ct = load(wp, "W1ct", eng=nc.gpsimd)     # [128, 12000] f16

        # head weight batch 2
        bl3c = SR("bl3c_row")
        wl3cpk = load(wp, "Wl3c_pk", eng=nc.sync)
        bcat1 = SR("bcat1_row")
        wcat1pk = load(wp, "Wcat1_pk", eng=nc.sync)
        wcat2pk = S32b("wcat2_pk")
        bcat2 = SR("bcat2_t")

        # ---- layer 3 -> z = c3 @ Wc4 [512, 3] -> AG-z ----
        layer_tail(agg_blocks(hload2(h_full[1])), wc4, bc_rep[2], z_slice,
                   ncols=3)
        allgather(z_slice[0:CLL_NPC, 0:3], z_full.ap())

        # ---- fusion head minis (fill the AG-z window) ----
        def mm_chain(p_rows, n_cols, k_steps, act_bias, out_tag):
            acc = sb.tile([p_rows, n_cols], f32, tag=out_tag + "a")
            for k in range(k_steps):
                lhsT, rhs = yield k
                pst = psM.tile([p_rows, n_cols], f32, tag="m", space="PSUM")
                for och in range(n_cols):
                    nc.tensor.matmul(pst[:, och:och + 1], lhsT(och), rhs,
                                     start=True, stop=True)
                if k == 0:
                    nc.vector.tensor_copy(acc[:], pst[:])
                else:
                    nc.vector.tensor_tensor(acc[:], acc[:], pst[:], op=ADD)
            o = sb.tile([p_rows, n_cols], f32, tag=out_tag, bufs=1)
            for och in range(n_cols):
                nc.scalar.activation(o[:, och:och + 1], acc[:, och:och + 1],
                                     RELU, bias=act_bias[:, och:och + 1])
            yield o

        def run_chain(p_rows, n_cols, pieces, act_bias, out_tag):
            gen = mm_chain(p_rows, n_cols, len(pieces), act_bias, out_tag)
            k = next(gen)
            while True:
                r = gen.send(pieces[k])
                if not isinstance(r, int):
                    return r
                k = r

        bvec = run_chain(P, 1, [
            (lambda o: wlb[0][:, 0:128], bgc0[:]),
            (lambda o: wlb[1][0:F - P, 0:128], bgc1[0:F - P, :]),
        ], blb, "bvec")

        d1 = run_chain(125, 4, [
            (lambda o: wd1[0][:, o * 125:(o + 1) * 125], mvec[:]),
            (lambda o: wd1[1][:, o * 125:(o + 1) * 125], bvec[:]),
        ], bd1, "d1")

        d2 = run_chain(P, 2, [
            (lambda o, k=k: wd2t[k][:, o * P:(o + 1) * P], d1[:, k:k + 1])
            for k in range(4)
        ], bd2, "d2")

        # ---- P^T = sum_s z_s^T Q_s, h4, dense-1 partials ----
        zf = sb.tile([P, CLL_NCH, 3], f16, tag="zf", bufs=1)
        nc.sync.dma_start(
            zf[:], z_full.ap()[0:CLL_NF, :].rearrange("(c p) f -> p c f", p=P))
        ptz = psM.tile([3, CLL_NPC], f32, tag="m", space="PSUM")
        for s in range(CLL_NCH):
            pi = piece_of(s)
            c0 = (s - PIECES[pi][0]) * CLL_NBLK * P
            nc.tensor.matmul(ptz[:], zf[:, s, 0:3],
                             qt_p[pi][:, c0:c0 + CLL_NBLK * P],
                             start=(s == 0), stop=(s == CLL_NCH - 1))
        ptz_sb = sb.tile([3, CLL_NPC], f32, tag="ptzsb", bufs=1)
        nc.vector.tensor_copy(ptz_sb[:], ptz[:])

        h4c = []
        for b in range(CLL_NBLK):
            pt4 = psT.tile([P, P], f32, tag="tp", space="PSUM")
            nc.tensor.transpose(pt4[0:P, 0:3],
                                ptz_sb[0:3, b * P:(b + 1) * P],
                                ident32[0:3, 0:3])
            th4 = sb.tile([P, 3], f32, tag="th4")
            nc.vector.tensor_tensor(th4[:], pt4[0:P, 0:3], bc4r[:, 0:3],
                                    op=ADD)
            hb = sb.tile([P, 3], f16, tag=f"h4c{b}", bufs=1)
            nc.scalar.activation(hb[:], th4[:], RELU)
            h4c.append(hb)

        # dense-1 partials, block-major so mms start as soon as h4c[0] lands
        JORDER = [ch * 4 + b for b in range(CLL_NBLK) for ch in range(3)]
        dsum = sb.tile([1, 1024], f32, tag="rowb", bufs=1)
        nc.vector.memset(dsum[0:1, 1000:1024], 0.0)
        for half in range(2):
            psd = psM.tile([1, 500], f32, tag="m", space="PSUM")
            for pos, j in enumerate(JORDER):
                ch, b = j // 4, j % 4
                nc.tensor.matmul(psd[:], h4c[b][:, ch:ch + 1],
                                 w1ct[:, j * 1000 + half * 500:
                                      j * 1000 + half * 500 + 500],
                                 start=(pos == 0), stop=(pos == 11))
            nc.vector.tensor_copy(dsum[0:1, half * 500:half * 500 + 500],
                                  psd[0:1, :])
        nc.gpsimd.dma_start(ar_in.ap()[0:1024, None], dsum[0:1, :])

        nc.gpsimd.collective_compute(
            "AllReduce", mybir.AluOpType.add, replica_groups=RG,
            ins=[ar_in.ap()], outs=[ar_out.ap()])

        # ---- fusion head (replicated) ----
        c1in = sb.tile([P, 8], f32, tag="c1in", bufs=1)
        nc.sync.dma_start(c1in[:], ar_out.ap().rearrange("(j p) -> p j", p=P))
        c1t = sb.tile([P, 8], f32, tag="c1t", bufs=1)
        nc.vector.tensor_tensor(c1t[:], c1in[:], bl1c[:], op=ADD)
        c1h = sb.tile([P, 8], f16, tag="c1h", bufs=1)
        nc.scalar.activation(c1h[:], c1t[:], RELU)

        def rowstage(lhs_cols, rhs_pk, rhs_cw, ncols, bias_row, tag,
                     out_f16=True):
            """out_row[1, ncols(+pad)] = relu(sum_j lhs_cols[j]^T rhs_j + b)."""
            npad = max(ncols, 1024) if ncols > 512 else ncols
            rb = sb.tile([1, npad], f32, tag="rowb", bufs=1)
            if npad > ncols:
                nc.vector.memset(rb[0:1, ncols:npad], 0.0)
            for h0 in range(0, ncols, 500):
                hw = min(500, ncols - h0)
                psr = psM.tile([1, hw], f32, tag="m", space="PSUM")
                for j, col in enumerate(lhs_cols):
                    nc.tensor.matmul(psr[:], col,
                                     rhs_pk[:, j * rhs_cw + h0:
                                            j * rhs_cw + h0 + hw],
                                     start=(j == 0),
                                     stop=(j == len(lhs_cols) - 1))
                nc.vector.tensor_tensor(rb[0:1, h0:h0 + hw], psr[0:1, :],
                                        bias_row[0:1, h0:h0 + hw], op=ADD)
            ro = sb.tile([1, npad], f16 if out_f16 else f32, tag=tag + "o",
                         bufs=1)
            nc.scalar.activation(ro[0:1, :], rb[0:1, :], RELU)
            return ro

        def rowcols(row, n, tag, idf):
            cols = sb.tile([P, n], row.dtype, tag=tag, bufs=1)
            for j in range(n):
                ptj = psT.tile([P, P], row.dtype, tag="tp", name="ptj",
                               space="PSUM")
                nc.tensor.transpose(ptj[0:P, 0:1], row[0:1, j * P:(j + 1) * P],
                                    idf[0:1, 0:1])
                nc.vector.tensor_copy(cols[:, j:j + 1], ptj[0:P, 0:1])
            return cols

        c1cols = [c1h[:, j:j + 1] for j in range(8)]
        c2h = rowstage(c1cols, wl2cpk, 1000, 1000, bl2c, "c2")
        c2c = rowcols(c2h, 8, "c2c", ident16)
        c3h = rowstage([c2c[:, j:j + 1] for j in range(8)], wl3cpk, 256, 256,
                       bl3c, "c3")
        d2h = sb.tile([P, 2], f16, tag="d2h", bufs=1)
        nc.vector.tensor_copy(d2h[:], d2[:])
        c3c = rowcols(c3h, 2, "c3c", ident16)
        ucols_in = [d2h[:, 0:1], d2h[:, 1:2], c3c[:, 0:1], c3c[:, 1:2]]
        uact = rowstage(ucols_in, wcat1pk, 1000, 1000, bcat1, "u",
                        out_f16=False)
        ucols = rowcols(uact, 8, "ucols", ident32)
        pso = psM.tile([1, 1], f32, tag="m", space="PSUM")
        for k in range(8):
            nc.tensor.matmul(pso[:], ucols[:, k:k + 1], wcat2pk[:, k:k + 1],
                             start=(k == 0), stop=(k == 7))
        osb = sb.tile([1, 1], f32, tag="osb", bufs=1)
        nc.scalar.activation(osb[:], pso[:], RELU, bias=bcat2[:])
        nc.sync.dma_start(out[0:1, 0:1], osb[:])


# ------------------------------------------------------------------- entry

_CACHE = {}


def kernel(**inputs):
    in_maps, meta = prep_inputs(inputs)
    if "nc" not in _CACHE:
        _CACHE["nc"] = build_program(meta)
    nc = _CACHE["nc"]
    res = run_bass_kernel_spmd(nc, in_maps, core_ids=list(range(NCORES)))
    return np.asarray(res.results[0]["out"], np.float32)


# revision 37
# speedup vs baseline: 1.1708x; 1.0531x over previous
"""Trainium2 Bass kernel for nn_DrugRank (GNN message passing), 8 NeuronCores.

Architecture (v3 — replicated L1 transform, z-trick tail):

  - Bio branch pruned to the 2-hop in-neighborhood of node 49999 (the only
    row the reference consumes); replicated per core.
  - cll graph (3451 nodes, 55216 edges, 4 GCN layers): dst-node sharded,
    512 nodes (4 blocks of 128) per core. GCN normalization folded host-side
    into dense per-(src-chunk, dst-block) adjacency tiles Q.
  - Layer 1: h1 = x_cll @ Wc1 is collective-free (x replicated input), so
    every core computes the FULL h1 (27 chunks) locally — no AllGather and
    the one-time CC barrier (~30-50us) overlaps this compute instead of
    idling the PE.
  - Layers 1-3 aggregate via 27x4 PSUM-accumulated 128x128x200 matmuls;
    per-layer AllGather of the 200KB transformed slice (AG2, AG3 only).
  - Layer 4 via associativity: (Q^T c3) @ Wc4 = Q^T (c3 @ Wc4). Each core
    computes z = c3_own @ Wc4 [512,3], AllGathers 3KB instead of 204KB,
    then P^T = sum_s z_s^T Q_s as 27 skinny matmuls. Saves AG4's wire time
    and its dead window.
  - mol branch, bio branch, and fusion-head mini-chains are emitted into
    the AG windows. Dense-1 row-sharded + AllReduce, head replicated.
"""

import numpy as np

import concourse.bacc as bacc
import concourse.bass as bass
import concourse.mybir as mybir
import concourse.tile as tile
from concourse.bass_utils import run_bass_kernel_spmd

NCORES = 8
P = 128
F = 200

CLL_N, CLL_E, CLL_PAD, CLL_NPC = 3451, 55216, 4096, 512
CLL_NBLK = CLL_NPC // P                 # 4 dst blocks / core
CLL_NCH = 27                            # src chunks with real nodes
CLL_NF = CLL_NCH * P                    # 3456 packed src nodes
N_BIO = 50000
BIO_S2, BIO_S1 = 768, 128               # padded bio 2-hop sets
BIO_NCH = BIO_S2 // P                   # 6
MOL_N, MOL_E = 64, 128

f32 = mybir.dt.float32
f16 = mybir.dt.float16
i16 = mybir.dt.int16
RELU = mybir.ActivationFunctionType.Relu
COPY = mybir.ActivationFunctionType.Copy
EQ = mybir.AluOpType.is_equal
MUL = mybir.AluOpType.mult
ADD = mybir.AluOpType.add

# Small weights are packed into a few blob tensors (one DMA each) because
# every dma_start costs ~0.6-1.3us of engine-queue time and the HWDGE
# completion-tracking window stalls the queue after ~10 outstanding DMAs.
# Each entry: (name, rows, cols); rows are zero-padded to the blob height.
BLOB16_SPEC = [
    ("wc2a", 128, F), ("wc2b", 72, F),
    ("wc3a", 128, F), ("wc3b", 72, F),
    ("wc4a", 128, 3), ("wc4b", 72, 3),
    ("wb2a", 128, F), ("wb2b", 72, F),
    ("qb1", 128, BIO_NCH * BIO_S1), ("qb2", 128, 1),
    ("xbT0", 128, BIO_S2), ("xbT1", 128, BIO_S2),
    ("wb1_0", 128, F), ("wb1_1", 128, F),
    ("ident16", 128, 128),
]
BLOB32E_SPEC = [
    ("bc1_rep", 128, F), ("bc2_rep", 128, F), ("bc3_rep", 128, F),
    ("bc4_rep", 128, 4),
    ("iota32", 128, 128), ("ident32", 128, 128), ("ones32", 128, 1),
    ("mol_slot", 128, 1),
]
BLOB32A_SPEC = [
    ("xmolT", 64, MOL_N),
    ("wm1r", 64, F), ("wm1s", 64, F),
    ("bm1_rep", 64, F), ("bm2_rep", 64, F),
]
BLOB32B_SPEC = [
    ("bb1_rep", 128, F),
    ("wm2ra", 128, F), ("wm2rb", 72, F),
    ("wm2sa", 128, F), ("wm2sb", 72, F),
    ("wlma", 128, 128), ("wlmb", 72, 128),
    ("wlba", 128, 128), ("wlbb", 72, 128),
    ("wd1_0", 128, 500), ("wd1_1", 128, 500),
    ("wd2_0", 125, 256), ("wd2_1", 125, 256),
    ("wd2_2", 125, 256), ("wd2_3", 125, 256),
    ("bd1_t", 125, 4), ("bd2_t", 128, 2),
    ("blm_col", 128, 1), ("blb_col", 128, 1),
    ("bl1c_pk", 128, 8), ("wcat2_pk", 128, 8),
]
BROW_SPEC = [
    ("bb2_row", 1, F), ("bl2c_row", 1, 1024), ("bl3c_row", 1, 256),
    ("bcat1_row", 1, 1024), ("bcat2_t", 1, 1),
]


def _blob_offsets(spec):
    offs, c = {}, 0
    for name, rows, cols in spec:
        offs[name] = (c, rows, cols)
        c += cols
    return offs, c


BLOB16_OFF, BLOB16_N = _blob_offsets(BLOB16_SPEC)
BLOB32E_OFF, BLOB32E_N = _blob_offsets(BLOB32E_SPEC)
BLOB32A_OFF, BLOB32A_N = _blob_offsets(BLOB32A_SPEC)
BLOB32B_OFF, BLOB32B_N = _blob_offsets(BLOB32B_SPEC)
BROW_OFF, BROW_N = _blob_offsets(BROW_SPEC)
_BLOBS = [("blob16", BLOB16_OFF), ("blob32e", BLOB32E_OFF),
          ("blob32a", BLOB32A_OFF), ("blob32b", BLOB32B_OFF),
          ("brow", BROW_OFF)]


def _pack_blob(spec, parts, height, dt):
    offs, total = _blob_offsets(spec)
    blob = np.zeros((height, total), dt)
    for name, rows, cols in spec:
        a = np.asarray(parts[name])
        assert a.shape == (rows, cols), (name, a.shape, rows, cols)
        blob[:rows, offs[name][0]:offs[name][0] + cols] = a
    return np.ascontiguousarray(blob)


def blob_get(m, name):
    """Extract an unpadded sub-array from the packed blobs (test helper)."""
    for key, offs in _BLOBS:
        if name in offs:
            c0, rows, cols = offs[name]
            return np.asarray(m[key])[0:rows, c0:c0 + cols]
    raise KeyError(name)


class Slab:
    """Column window of a blob tile, sliceable like a standalone tile."""

    def __init__(self, tile, off, rows, cols):
        self.t, self.off, self.rows, self.cols = tile, off, rows, cols

    def __getitem__(self, key):
        if not isinstance(key, tuple):
            key = (key, slice(None))
        rs, cs = key
        r0 = rs.start if rs.start is not None else 0
        r1 = rs.stop if rs.stop is not None else self.rows
        c0 = cs.start if cs.start is not None else 0
        c1 = cs.stop if cs.stop is not None else self.cols
        return self.t[r0:r1, self.off + c0:self.off + c1]


# ---------------------------------------------------------------- host prep

def _pack_idx16(flat):
    n = len(flat)
    a16 = np.asarray(flat, np.int16).reshape(n // 16, 16).T
    return np.ascontiguousarray(np.tile(a16, (8, 1)))


def _pack_slots(flat, dtype=np.float16):
    n = len(flat)
    return np.ascontiguousarray(
        np.asarray(flat, np.float64).astype(dtype).reshape(n // P, P).T)


def _col(v):
    return np.ascontiguousarray(np.asarray(v, np.float32).reshape(-1, 1))


def _rep(v, rows=P):
    return np.ascontiguousarray(
        np.tile(np.asarray(v, np.float32).reshape(1, -1), (rows, 1)))


def _btile(v, p, n):
    return np.ascontiguousarray(np.asarray(v, np.float32).reshape(n, p).T)


def _rowpad(v, n):
    """[m] -> [1, n] zero-padded row."""
    v = np.asarray(v, np.float32).reshape(-1)
    o = np.zeros((1, n), np.float32)
    o[0, :len(v)] = v
    return o


def _rowpack(w, rows_pad, cols, dt=np.float16):
    """[m, cols] -> [128, (rows_pad//128)*cols]: [p, j*cols+q] = w[j*128+p, q]."""
    w = np.asarray(w, np.float32)
    wp = np.zeros((rows_pad, cols), np.float32)
    wp[:w.shape[0]] = w
    nj = rows_pad // P
    return np.ascontiguousarray(
        wp.reshape(nj, P, cols).transpose(1, 0, 2).reshape(P, nj * cols)
    ).astype(dt)


def _cll_q(edge, dinv):
    """Dense normalized adjacency, [CLL_PAD, CLL_PAD] f32."""
    src = edge[0].astype(np.int64)
    dst = edge[1].astype(np.int64)
    q = np.zeros((CLL_PAD, CLL_PAD), np.float32)
    np.add.at(q, (src, dst), (dinv[src] * dinv[dst]).astype(np.float32))
    di = np.arange(CLL_N)
    q[di, di] += (dinv[:CLL_N] * dinv[:CLL_N]).astype(np.float32)
    return q


def _bio_prune(edge, x_bio):
    """2-hop in-neighborhood of node N_BIO-1 -> (xbT_sel, Qb1_pk, Qb2)."""
    src = edge[0].astype(np.int64)
    dst = edge[1].astype(np.int64)
    deg = np.bincount(dst, minlength=N_BIO).astype(np.float64) + 1.0
    dinv = 1.0 / np.sqrt(deg)
    tgt = N_BIO - 1

    m2 = dst == tgt
    s1 = np.unique(np.concatenate([src[m2], [tgt]]))
    assert len(s1) <= BIO_S1, len(s1)
    pos1 = np.full(N_BIO, -1, np.int64)
    pos1[s1] = np.arange(len(s1))

    m1 = pos1[dst] >= 0
    e1s, e1d = src[m1], dst[m1]
    s2 = np.unique(np.concatenate([e1s, s1]))
    assert len(s2) <= BIO_S2, len(s2)
    pos2 = np.full(N_BIO, -1, np.int64)
    pos2[s2] = np.arange(len(s2))

    q1 = np.zeros((BIO_S2, BIO_S1), np.float32)
    np.add.at(q1, (pos2[e1s], pos1[e1d]),
              (dinv[e1s] * dinv[e1d]).astype(np.float32))
    q1[pos2[s1], pos1[s1]] += (dinv[s1] * dinv[s1]).astype(np.float32)

    q2 = np.zeros((BIO_S1, 1), np.float32)
    np.add.at(q2, (pos1[src[m2]], 0),
              (dinv[src[m2]] * dinv[tgt]).astype(np.float32))
    q2[pos1[tgt], 0] += np.float32(dinv[tgt] * dinv[tgt])

    xsel = np.zeros((BIO_S2, 256), np.float32)
    xsel[:len(s2)] = x_bio[s2]
    xbT = np.ascontiguousarray(xsel.T).astype(np.float16)      # [256, 768]
    # Qb1 packed [128, 6*128]: [p, s*128+d] = q1[s*128+p, d]
    q1pk = np.ascontiguousarray(
        q1.reshape(BIO_NCH, P, BIO_S1).transpose(1, 0, 2)
        .reshape(P, BIO_NCH * BIO_S1)).astype(np.float16)
    return xbT, q1pk, q2.astype(np.float16)


def prep_inputs(inp):
    meta = {}
    # ---- cll Q tiles ----
    dst = inp["edge_cll"][1].astype(np.int64)
    deg = np.bincount(dst, minlength=CLL_N).astype(np.float64) + 1.0
    dinv = np.zeros(CLL_PAD, np.float64)
    dinv[:CLL_N] = 1.0 / np.sqrt(deg)
    q = _cll_q(inp["edge_cll"], dinv)

    xcT = np.zeros((512, CLL_NF), np.float32)
    xcT[:, :CLL_N] = inp["x_cll"].T
    # full-graph xcllT packed [128, 4*3456]: [p, k*3456+n] = x_cll.T[k*128+p, n]
    xcllT_full = np.ascontiguousarray(
        xcT.reshape(4, P, CLL_NF).transpose(1, 0, 2)
        .reshape(P, 4 * CLL_NF)).astype(np.float16)

    # W1c regrouped: rows (node*3+ch) -> per core [128, 12*1000] f16,
    # col-block j = ch*4+blk, rows = local node p of that block.
    w1c = np.asarray(inp["Wl1c"], np.float32)                  # [10353, 1000]
    w1c_n = np.zeros((CLL_PAD, 3, 1000), np.float32)
    w1c_n[:CLL_N] = w1c.reshape(CLL_N, 3, 1000)

    xbT_sel, q1pk, q2 = _bio_prune(inp["edge_bio"], np.asarray(inp["x_bio"]))

    mol_s = inp["edge_mol"][0].astype(np.int64)
    mol_d = inp["edge_mol"][1].astype(np.int64)
    order = np.argsort(mol_d, kind="stable")
    mol_idx = _pack_idx16(mol_s[order])
    mol_slot = _pack_slots(mol_d[order].astype(np.float64), np.float32)

    iota = np.tile(np.arange(P, dtype=np.float32), (P, 1))
    ident = np.eye(P, dtype=np.float32)

    wc1 = np.asarray(inp["Wc1"], np.float32)                   # [512, 200]

    def _f16(x):
        return np.asarray(x, np.float16)

    def _f32(x):
        return np.asarray(x, np.float32)

    wb2 = _f16(inp["Wb2"])
    wm2r, wm2s = _f32(inp["Wm2r"]), _f32(inp["Wm2s"])
    wlm, wlb = _f32(inp["Wlm"]), _f32(inp["Wlb"])
    wd1, wd2 = _f32(inp["Wd1"]), _f32(inp["Wd2"])
    p16 = {
        "wc2a": _f16(inp["Wc2"])[:128], "wc2b": _f16(inp["Wc2"])[128:],
        "wc3a": _f16(inp["Wc3"])[:128], "wc3b": _f16(inp["Wc3"])[128:],
        "wc4a": _f16(inp["Wc4"])[:128], "wc4b": _f16(inp["Wc4"])[128:],
        "wb2a": wb2[:128], "wb2b": wb2[128:],
        "qb1": q1pk, "qb2": np.tile(q2, (1, 1)),
        "xbT0": xbT_sel[:128], "xbT1": xbT_sel[128:],
        "wb1_0": _f16(inp["Wb1"])[:128], "wb1_1": _f16(inp["Wb1"])[128:],
        "ident16": ident.astype(np.float16),
    }
    p32e = {
        "bc1_rep": _rep(inp["bc1"]), "bc2_rep": _rep(inp["bc2"]),
        "bc3_rep": _rep(inp["bc3"]),
        "bc4_rep": np.pad(_rep(inp["bc4"]), ((0, 0), (0, 1))),
        "iota32": iota, "ident32": ident,
        "ones32": np.ones((P, 1), np.float32),
        "mol_slot": mol_slot,
    }
    p32a = {
        "xmolT": np.ascontiguousarray(inp["x_mol"].T.astype(np.float32)),
        "wm1r": _f32(inp["Wm1r"]), "wm1s": _f32(inp["Wm1s"]),
        "bm1_rep": _rep(inp["bm1"], 64), "bm2_rep": _rep(inp["bm2"], 64),
    }
    p32b = {
        "bb1_rep": _rep(inp["bb1"]),
        "wm2ra": wm2r[:128], "wm2rb": wm2r[128:],
        "wm2sa": wm2s[:128], "wm2sb": wm2s[128:],
        "wlma": wlm[:128], "wlmb": wlm[128:],
        "wlba": wlb[:128], "wlbb": wlb[128:],
        "wd1_0": wd1[:128], "wd1_1": wd1[128:],
        "wd2_0": wd2[0:125], "wd2_1": wd2[125:250],
        "wd2_2": wd2[250:375], "wd2_3": wd2[375:500],
        "bd1_t": _btile(inp["bd1"], 125, 4),
        "bd2_t": _btile(inp["bd2"], 128, 2),
        "blm_col": _col(inp["blm"]), "blb_col": _col(inp["blb"]),
        "bl1c_pk": np.ascontiguousarray(
            _rowpad(inp["bl1c"], 1024).reshape(8, P).T),
        "wcat2_pk": _rowpack(inp["Wcat2"], 1024, 1, np.float32),
    }
    prow = {
        "bb2_row": np.ascontiguousarray(
            np.asarray(inp["bb2"], np.float32).reshape(1, -1)),
        "bl2c_row": _rowpad(inp["bl2c"], 1024),
        "bl3c_row": _rowpad(inp["bl3c"], 256),
        "bcat1_row": _rowpad(inp["bcat1"], 1024),
        "bcat2_t": np.asarray(inp["bcat2"], np.float32).reshape(1, 1),
    }
    shared = {
        "xcllT": xcllT_full,
        "Wc1": np.ascontiguousarray(
            wc1.reshape(4, P, F).transpose(1, 0, 2)
            .reshape(P, 4 * F)).astype(np.float16),
        "x_mol": np.asarray(inp["x_mol"], np.float32),
        "mol_idx": mol_idx,
        "Wcat1_pk": _rowpack(inp["Wcat1"], 512, 1000),
        "Wl2c_pk": _rowpack(inp["Wl2c"], 1024, 1000),
        "Wl3c_pk": _rowpack(inp["Wl3c"], 1024, 256),
        "blob16": _pack_blob(BLOB16_SPEC, p16, P, np.float16),
        "blob32e": _pack_blob(BLOB32E_SPEC, p32e, P, np.float32),
        "blob32a": _pack_blob(BLOB32A_SPEC, p32a, P, np.float32),
        "blob32b": _pack_blob(BLOB32B_SPEC, p32b, P, np.float32),
        "brow": _pack_blob(BROW_SPEC, prow, 1, np.float32),
    }
    in_maps = []
    for c in range(NCORES):
        m = dict(shared)
        lo = c * CLL_NPC
        # Qt packed [128, 27*4*128]: [p, (s*4+b)*128+d] = q[s*128+p, lo+b*128+d]
        qc = q[:CLL_NF, lo:lo + CLL_NPC]
        m["Qt"] = np.ascontiguousarray(
            qc.reshape(CLL_NCH, P, CLL_NBLK, P).transpose(1, 0, 2, 3)
            .reshape(P, CLL_NCH * CLL_NBLK * P)).astype(np.float16)
        # W1ct [128, 12*1000]: [p, (ch*4+blk)*1000+q] = w1c_n[lo+blk*128+p, ch, q]
        wslice = w1c_n[lo:lo + CLL_NPC]                         # [512, 3, 1000]
        m["W1ct"] = np.ascontiguousarray(
            wslice.reshape(CLL_NBLK, P, 3, 1000).transpose(1, 2, 0, 3)
            .reshape(P, 12 * 1000)).astype(np.float16)
        in_maps.append(m)
    return in_maps, meta


# ------------------------------------------------------------ device program

RG = [list(range(NCORES))]


def _declare_inputs(nc):
    spec = {
        "xcllT": ([P, 4 * CLL_NF], f16),
        "Qt": ([P, CLL_NCH * CLL_NBLK * P], f16),
        "Wc1": ([P, 4 * F], f16),
        "W1ct": ([P, 12 * 1000], f16),
        "x_mol": ([MOL_N, 64], f32),
        "mol_idx": ([P, 8], i16),
        "Wcat1_pk": ([P, 4 * 1000], f16),
        "Wl2c_pk": ([P, 8 * 1000], f16),
        "Wl3c_pk": ([P, 8 * 256], f16),
        "blob16": ([P, BLOB16_N], f16),
        "blob32e": ([P, BLOB32E_N], f32),
        "blob32a": ([P, BLOB32A_N], f32),
        "blob32b": ([P, BLOB32B_N], f32),
        "brow": ([1, BROW_N], f32),
    }
    return {k: nc.dram_tensor(k, s, d, kind="ExternalInput")
            for k, (s, d) in spec.items()}


def build_program(meta=None, repeat=1):
    nc = bacc.Bacc("TRN2", target_bir_lowering=False, debug=False,
                   enable_asserts=False, num_devices=NCORES,
                   num_swdge_queues=4)
    io = _declare_inputs(nc)
    out = nc.dram_tensor("out", [1, 1], f32, kind="ExternalOutput")

    # h_slice/h_full for layers 2 and 3 (AG2, AG3, each split in 2 halves);
    # z for the layer-4 trick
    h_slice = [nc.dram_tensor(f"h{l}_slice", [CLL_NPC, F], f16,
                              kind="Internal") for l in range(2)]
    h_full = [nc.dram_tensor(f"h{l}_full", [CLL_PAD, F], f16,
                             kind="Internal", addr_space="Shared")
              for l in range(2)]
    z_slice = nc.dram_tensor("z_slice", [CLL_NPC, 3], f16, kind="Internal")
    z_full = nc.dram_tensor("z_full", [CLL_PAD, 3], f16, kind="Internal",
                            addr_space="Shared")
    bar_in = nc.dram_tensor("bar_in", [8], f32, kind="Internal")
    bar_out = nc.dram_tensor("bar_out", [8], f32, kind="Internal",
                             addr_space="Shared")
    m1_dram = nc.dram_tensor("m1_dram", [MOL_N, 256], f32, kind="Internal")
    ar_in = nc.dram_tensor("ar_in", [1024], f32, kind="Internal")
    ar_out = nc.dram_tensor("ar_out", [1024], f32, kind="Internal",
                            addr_space="Shared")

    with tile.TileContext(nc) as tc:
        for _ in range(repeat):
            _build(nc, tc, io, out, h_slice, h_full, z_slice, z_full,
                   m1_dram, ar_in, ar_out, bar_in, bar_out)
    nc.compile()
    return nc


def _build(nc, tc, io, out, h_slice, h_full, z_slice, z_full,
           m1_dram, ar_in, ar_out, bar_in, bar_out):
    with (
        tc.tile_pool(name="const", bufs=1) as cp,
        tc.tile_pool(name="wp", bufs=1) as wp,
        tc.tile_pool(name="hp", bufs=2) as hp,
        tc.tile_pool(name="sb", bufs=2) as sb,
        tc.tile_pool(name="ct", bufs=1) as ctp,
        tc.tile_pool(name="psA", bufs=1, space="PSUM") as psA,
        tc.tile_pool(name="psT", bufs=2, space="PSUM") as psT,
        tc.tile_pool(name="psM", bufs=2, space="PSUM") as psM,
    ):
        def load(pool, name, rows=None, cols=None, tag=None, dt=None,
                 eng=None):
            src = io[name]
            r = rows if rows is not None else src.shape[0]
            c = cols if cols is not None else src.shape[1]
            t = pool.tile([r, c], dt or src.dtype, tag=tag or name)
            (eng or nc.sync).dma_start(t[:], src[0:r, 0:c])
            return t

        def load2(name, tag, rows=F, cols=F, eng=None):
            """[rows>128, cols] -> two tiles [128, cols] + [rows-128, cols]."""
            a = load(wp, name, rows=P, cols=cols, tag=tag + "a", eng=eng)
            b = wp.tile([P, cols], io[name].dtype, tag=tag + "b")
            (eng or nc.sync).dma_start(b[0:rows - P, :], io[name][P:rows, 0:cols])
            return a, b

        def loadrows(name, nparts, cols, tag, rows=P, eng=None):
            """Tall [nparts*rows?, cols] tensor -> list of [128, cols] tiles."""
            ts = []
            for k in range(nparts):
                t = wp.tile([rows, cols], io[name].dtype, tag=f"{tag}{k}")
                (eng or nc.sync).dma_start(
                    t[:], io[name][k * rows:(k + 1) * rows, 0:cols])
                ts.append(t)
            return ts

        # ---- phase A inputs first: Wc1 + xcllT (8 half-chunk tiles over
        # the two HWDGE queues, first halves first), Qt pieces on SWDGE ----
        nc.gpsimd.collective_compute(
            "AllReduce", mybir.AluOpType.add, replica_groups=RG,
            ins=[bar_in.ap()], outs=[bar_out.ap()])
        wc1 = load(wp, "Wc1", eng=nc.scalar)       # [128, 4*200] f16, k-major
        b32e = load(wp, "blob32e", eng=nc.scalar)  # early-critical constants

        PIECES = [(0, 7), (7, 14), (14, 21), (21, CLL_NCH)]

        # xcllT tiled per (k, piece) and Qt per piece, all loads round-robin
        # over the 3 DMA queues in CONSUMPTION order, so the per-queue
        # ~100GB/s arrival front-runs the fused transform+aggregate loop.
        ENGS = [nc.sync, nc.scalar, nc.gpsimd]
        xckp = [[wp.tile([P, (s1 - s0) * P], f16, tag=f"xc{k}_{pi}",
                         name=f"xc{k}_{pi}")
                 for pi, (s0, s1) in enumerate(PIECES)] for k in range(4)]
        qt_p = [wp.tile([P, (s1 - s0) * CLL_NBLK * P], f16, tag=f"Qt{pi}",
                        name=f"Qt{pi}")
                for pi, (s0, s1) in enumerate(PIECES)]
        ei = 0
        for pi, (s0, s1) in enumerate(PIECES):
            for k in range(4):
                ENGS[ei % 3].dma_start(
                    xckp[k][pi][:],
                    io["xcllT"][:, k * CLL_NF + s0 * P:k * CLL_NF + s1 * P])
                ei += 1
            ENGS[ei % 3].dma_start(
                qt_p[pi][:],
                io["Qt"][:, s0 * CLL_NBLK * P:s1 * CLL_NBLK * P])
            ei += 1

        def piece_of(s):
            return next(i for i, (a, b) in enumerate(PIECES) if a <= s < b)

        def qtcol(s, b):
            pi = piece_of(s)
            c0 = ((s - PIECES[pi][0]) * CLL_NBLK + b) * P
            return qt_p[pi][:, c0:c0 + P]

        # ---- phase A fused with L1 aggregation: per piece, transform
        # h1 chunks then immediately aggregate them, so the agg matmuls of
        # piece p hide the xcllT DMA tail for piece p+1 ----
        h1p = []
        for pi, (s0, s1) in enumerate(PIECES):
            h1p.append(hp.tile([P, s1 - s0, F], f16, tag=f"hft{pi}",
                               name=f"hft{pi}"))
        h1pss = [psA.tile([P, F], f32, tag=f"agg{b}", name=f"agg{b}",
                          space="PSUM") for b in range(CLL_NBLK)]
        for pi, (s0, s1) in enumerate(PIECES):
            for s in range(s0, s1):
                ps = psM.tile([P, F], f32, tag="m", space="PSUM")
                for k in range(4):
                    nc.tensor.matmul(
                        ps[:], xckp[k][pi][:, (s - s0) * P:(s - s0 + 1) * P],
                        wc1[:, k * F:(k + 1) * F],
                        start=(k == 0), stop=(k == 3))
                nc.vector.tensor_copy(h1p[pi][:, s - s0, :], ps[:])
            for s in range(s0, s1):
                for b in range(CLL_NBLK):
                    nc.tensor.matmul(h1pss[b][:], qtcol(s, b),
                                     h1p[pi][:, s - s0, 0:F],
                                     start=(s == 0), stop=(s == CLL_NCH - 1))

        # blob loads: one DMA each (trigger cost + the HWDGE completion
        # window make many small DMAs poisonous), then early mol prep
        mol_idx_sb = load(cp, "mol_idx", eng=nc.scalar)
        b16 = load(wp, "blob16", eng=nc.scalar)
        browt = load(wp, "brow", eng=nc.scalar)
        b32a = load(wp, "blob32a", eng=nc.sync)
        b32b = load(wp, "blob32b", eng=nc.sync)

        def _slab(tile, offs, name):
            c0, r, c = offs[name]
            return Slab(tile, c0, r, c)

        def S16(n):
            return _slab(b16, BLOB16_OFF, n)

        def S32a(n):
            return _slab(b32a, BLOB32A_OFF, n)

        def S32e(n):
            return _slab(b32e, BLOB32E_OFF, n)

        def S32b(n):
            return _slab(b32b, BLOB32B_OFF, n)

        def SR(n):
            return _slab(browt, BROW_OFF, n)

        iota32 = S32e("iota32")
        ident32 = S32e("ident32")
        ident16 = S16("ident16")
        ones32 = S32e("ones32")
        mol_slot_sb = S32e("mol_slot")
        xmolT_sb = S32a("xmolT")
        v1 = sb.tile([P, 1, 64], f32, tag="vm")
        nc.gpsimd.dma_gather(v1[:], io["x_mol"].ap(), mol_idx_sb[:],
                             MOL_E, MOL_E, 64)

        wc2 = (S16("wc2a"), S16("wc2b"))
        bc_rep = [S32e("bc1_rep"), S32e("bc2_rep"), S32e("bc3_rep")]

        def allgather(src, dst):
            nc.gpsimd.collective_compute(
                "AllGather", mybir.AluOpType.bypass, replica_groups=RG,
                ins=[src], outs=[dst])

        def hload(hx):
            """h_full [4096, F] -> chunk_srcs [(tile, idx, s)] per piece."""
            srcs = []
            src = hx.ap().rearrange("(c p) f -> p c f", p=P)
            for pi, (s0, s1) in enumerate(PIECES):
                t = hp.tile([P, s1 - s0, F], f16, tag=f"hft{pi}",
                            name=f"hft{pi}")
                nc.sync.dma_start(t[:], src[:, s0:s1, :])
                srcs.extend((t, s - s0, s) for s in range(s0, s1))
            return srcs

        def transpose_to(src_sb, dst0, dst1, bcol):
            """src [128, 200] f32 -> dst0[128, bcol:+128], dst1[72, bcol:+128] f16."""
            pt = psT.tile([P, P], f32, tag="tp", space="PSUM")
            nc.tensor.transpose(pt[0:P, 0:P], src_sb[:, 0:P], ident32[:])
            nc.vector.tensor_copy(dst0[:, bcol:bcol + P], pt[0:P, 0:P])
            pt2 = psT.tile([P, P], f32, tag="tp", space="PSUM")
            nc.tensor.transpose(pt2[0:F - P, 0:P], src_sb[:, P:F], ident32[:])
            nc.vector.tensor_copy(dst1[0:F - P, bcol:bcol + P],
                                  pt2[0:F - P, 0:P])

        def agg_blocks(chunk_srcs):
            """4 PSUM accumulators over an arbitrary chunk processing order
            (starts as soon as the first source tile is available)."""
            pss = [psA.tile([P, F], f32, tag=f"agg{b}", name=f"agg{b}",
                            space="PSUM") for b in range(CLL_NBLK)]
            n = len(chunk_srcs)
            for pos, (t, idx, s) in enumerate(chunk_srcs):
                for b in range(CLL_NBLK):
                    nc.tensor.matmul(pss[b][:], qtcol(s, b),
                                     t[:, idx, 0:F],
                                     start=(pos == 0), stop=(pos == n - 1))
            return pss

        def layer_tail(pss, wnext, brep, dst_dram, ncols=F, ag_halves=None):
            """relu(+bias), transform by wnext ([128,c]+[72,c] tiles), store
            [512, ncols] f16 slices; optionally trigger the half-AllGathers
            after blocks 1 and 3."""
            cT0 = ctp.tile([P, CLL_NPC], f16, tag="cT0")
            cT1 = ctp.tile([P, CLL_NPC], f16, tag="cT1")
            for b in range(CLL_NBLK):
                t2 = sb.tile([P, F], f32, tag="ev1")
                nc.vector.tensor_tensor(t2[:], pss[b][:], brep[:], op=ADD)
                cblk = sb.tile([P, F], f32, tag="cblk", bufs=3)
                nc.scalar.activation(cblk[:], t2[:], RELU)
                transpose_to(cblk, cT0, cT1, b * P)
                wa, wb_ = wnext
                ph = psM.tile([P, ncols], f32, tag="m", space="PSUM")
                nc.tensor.matmul(ph[:], cT0[:, b * P:(b + 1) * P],
                                 wa[:, 0:ncols], start=True, stop=False)
                nc.tensor.matmul(ph[:], cT1[0:F - P, b * P:(b + 1) * P],
                                 wb_[0:F - P, 0:ncols], start=False, stop=True)
                hst = sb.tile([P, ncols], f16, tag="hst", bufs=3)
                nc.vector.tensor_copy(hst[:], ph[:])
                nc.sync.dma_start(dst_dram[b * P:(b + 1) * P, 0:ncols],
                                  hst[:])
                if ag_halves is not None and b == CLL_NBLK - 1:
                    allgather(dst_dram[0:CLL_NPC, 0:ncols], ag_halves.ap())

        # ---- layer 1 tail -> h2 -> AG2 ----
        layer_tail(h1pss, wc2, bc_rep[0], h_slice[0], ag_halves=h_full[0])

        # ---- bio mini-branch (fills the AG2/barrier window) ----
        xbT = [S16("xbT0"), S16("xbT1")]
        qb1 = S16("qb1")
        qb2 = S16("qb2")
        wb1 = [S16("wb1_0"), S16("wb1_1")]
        wb2 = (S16("wb2a"), S16("wb2b"))
        bb1r = S32b("bb1_rep")
        bb2row = SR("bb2_row")
        wc3 = (S16("wc3a"), S16("wc3b"))
        wc4 = (S16("wc4a"), S16("wc4b"))
        bc4r = S32e("bc4_rep")
        h1b = sb.tile([P, BIO_NCH, F], f16, tag="h1b", bufs=1)
        for j in range(BIO_NCH):
            ps = psM.tile([P, F], f32, tag="m", space="PSUM")
            for k in range(2):
                nc.tensor.matmul(ps[:], xbT[k][:, j * P:(j + 1) * P],
                                 wb1[k][:],
                                 start=(k == 0), stop=(k == 1))
            nc.vector.tensor_copy(h1b[:, j, :], ps[:])
        psb = psM.tile([P, F], f32, tag="m", space="PSUM")
        for j in range(BIO_NCH):
            nc.tensor.matmul(psb[:], qb1[:, j * P:(j + 1) * P], h1b[:, j, 0:F],
                             start=(j == 0), stop=(j == BIO_NCH - 1))
        tb1 = sb.tile([P, F], f32, tag="ev1")
        nc.vector.tensor_tensor(tb1[:], psb[:], bb1r[:], op=ADD)
        c1b = sb.tile([P, F], f32, tag="c1b", bufs=1)
        nc.scalar.activation(c1b[:], tb1[:], RELU)
        c1bT0 = sb.tile([P, P], f16, tag="c1bT0", bufs=1)
        c1bT1 = sb.tile([P, P], f16, tag="c1bT1", bufs=1)
        ptb = psT.tile([P, P], f32, tag="tp", space="PSUM")
        nc.tensor.transpose(ptb[0:P, 0:P], c1b[:, 0:P], ident32[:])
        nc.vector.tensor_copy(c1bT0[:], ptb[0:P, 0:P])
        ptb2 = psT.tile([P, P], f32, tag="tp", space="PSUM")
        nc.tensor.transpose(ptb2[0:F - P, 0:P], c1b[:, P:F], ident32[:])
        nc.vector.tensor_copy(c1bT1[0:F - P, :], ptb2[0:F - P, 0:P])
        ph2 = psM.tile([P, F], f32, tag="m", space="PSUM")
        nc.tensor.matmul(ph2[:], c1bT0[:, 0:P], wb2[0][:],
                         start=True, stop=False)
        nc.tensor.matmul(ph2[:], c1bT1[0:F - P, 0:P], wb2[1][0:F - P, :],
                         start=False, stop=True)
        h2b = sb.tile([P, F], f16, tag="h2b", bufs=1)
        nc.vector.tensor_copy(h2b[:], ph2[:])
        pr = psM.tile([1, F], f32, tag="m", space="PSUM")
        nc.tensor.matmul(pr[:], qb2[:], h2b[:], start=True, stop=True)
        tb2 = sb.tile([1, F], f32, tag="ev1")
        nc.vector.tensor_tensor(tb2[0:1, :], pr[0:1, :], bb2row[0:1, :], op=ADD)
        brow = sb.tile([1, F], f32, tag="brow", bufs=1)
        nc.scalar.activation(brow[0:1, :], tb2[0:1, :], RELU)
        # bvec column [200, 1] for the head
        bgc0 = sb.tile([P, 1], f32, tag="bgc0", bufs=1)
        bgc1 = sb.tile([P, 1], f32, tag="bgc1", bufs=1)
        prc = psT.tile([P, P], f32, tag="tp", space="PSUM")
        nc.tensor.transpose(prc[0:P, 0:1], brow[0:1, 0:P], ident32[0:1, 0:1])
        nc.vector.tensor_copy(bgc0[:], prc[0:P, 0:1])
        prc2 = psT.tile([P, P], f32, tag="tp", space="PSUM")
        nc.tensor.transpose(prc2[0:F - P, 0:1], brow[0:1, P:F],
                            ident32[0:1, 0:1])
        nc.vector.tensor_copy(bgc1[0:F - P, :], prc2[0:F - P, 0:1])

        # head weights (slabs) + big packed tables
        wlb = (S32b("wlba"), S32b("wlbb"))
        blb = S32b("blb_col")
        wd1 = (S32b("wd1_0"), S32b("wd1_1"))
        bd1 = S32b("bd1_t")
        wd2t = [S32b(f"wd2_{k}") for k in range(4)]
        bd2 = S32b("bd2_t")
        bl1c = S32b("bl1c_pk")
        bl2c = SR("bl2c_row")
        wl2cpk = load(wp, "Wl2c_pk", eng=nc.sync)

        # ---- mol branch part 1 + gather-2 trigger (SWDGE latency hides
        # under the AG2 window) ----
        wm1r = S32a("wm1r")
        wm1s = S32a("wm1s")
        bm1r = S32a("bm1_rep")
        bm2r = S32a("bm2_rep")
        mM = sb.tile([P, 64], f32, tag="Mmol", bufs=1)
        nc.vector.tensor_scalar(mM[:], iota32[:, 0:64], mol_slot_sb[:, 0:1],
                                None, op0=EQ)
        agg_ps = psM.tile([64, 64], f32, tag="m", space="PSUM")
        nc.tensor.matmul(agg_ps[:], mM[:], v1[:, 0, :], start=True, stop=True)
        agg_sb = sb.tile([64, 64], f32, tag="mol1")
        nc.vector.tensor_copy(agg_sb[:], agg_ps[:])
        pt = psT.tile([P, P], f32, tag="tp", space="PSUM")
        nc.tensor.transpose(pt[0:64, 0:64], agg_sb[0:64, 0:64],
                            ident32[0:64, 0:64])
        aggT = sb.tile([64, 64], f32, tag="mol2")
        nc.vector.tensor_copy(aggT[:], pt[0:64, 0:64])
        h1_ps = psM.tile([64, F], f32, tag="m", space="PSUM")
        nc.tensor.matmul(h1_ps[:], aggT[:], wm1r[:], start=True, stop=False)
        nc.tensor.matmul(h1_ps[:], xmolT_sb[:], wm1s[:], start=False, stop=True)
        t_m1 = sb.tile([64, F], f32, tag="mol3")
        nc.vector.tensor_tensor(t_m1[:], h1_ps[:], bm1r[0:64, :], op=ADD)
        m1_sb = sb.tile([64, F], f32, tag="mol4", bufs=1)
        nc.scalar.activation(m1_sb[:], t_m1[:], RELU)
        nc.gpsimd.dma_start(m1_dram[0:64, 0:F], m1_sb[:])

        wm2r = (S32b("wm2ra"), S32b("wm2rb"))
        wm2s = (S32b("wm2sa"), S32b("wm2sb"))
        v2 = sb.tile([P, 1, 256], f32, tag="vm2")
        nc.gpsimd.dma_gather(v2[:], m1_dram.ap(), mol_idx_sb[:],
                             MOL_E, MOL_E, 256)

        # ---- mol branch part 2 (fills the AG2 window) ----
        agg2_ps = psM.tile([64, F], f32, tag="m", space="PSUM")
        nc.tensor.matmul(agg2_ps[:], mM[:], v2[:, 0, 0:F], start=True, stop=True)
        agg2_sb = sb.tile([64, F], f32, tag="mol1")
        nc.vector.tensor_copy(agg2_sb[:], agg2_ps[:])
        a2T0 = sb.tile([P, 64], f32, tag="mol5")
        a2T1 = sb.tile([P, 64], f32, tag="mol6")
        m1T0 = sb.tile([P, 64], f32, tag="mol7")
        m1T1 = sb.tile([P, 64], f32, tag="mol8")
        for srcT, d0, d1 in ((agg2_sb, a2T0, a2T1), (m1_sb, m1T0, m1T1)):
            pt1 = psT.tile([P, P], f32, tag="tp", space="PSUM")
            nc.tensor.transpose(pt1[0:P, 0:64], srcT[0:64, 0:P],
                                ident32[0:64, 0:64])
            nc.vector.tensor_copy(d0[:, 0:64], pt1[0:P, 0:64])
            pt2 = psT.tile([P, P], f32, tag="tp", space="PSUM")
            nc.tensor.transpose(pt2[0:F - P, 0:64], srcT[0:64, P:F],
                                ident32[0:64, 0:64])
            nc.vector.tensor_copy(d1[0:F - P, 0:64], pt2[0:F - P, 0:64])
        h2_ps = psM.tile([64, F], f32, tag="m", space="PSUM")
        nc.tensor.matmul(h2_ps[:], a2T0[:, 0:64], wm2r[0][:],
                         start=True, stop=False)
        nc.tensor.matmul(h2_ps[:], a2T1[0:F - P, 0:64], wm2r[1][0:F - P, :],
                         start=False, stop=False)
        nc.tensor.matmul(h2_ps[:], m1T0[:, 0:64], wm2s[0][:],
                         start=False, stop=False)
        nc.tensor.matmul(h2_ps[:], m1T1[0:F - P, 0:64], wm2s[1][0:F - P, :],
                         start=False, stop=True)
        t_m2 = sb.tile([64, F], f32, tag="mol3")
        nc.vector.tensor_tensor(t_m2[:], h2_ps[:], bm2r[0:64, :], op=ADD)
        m2_sb = sb.tile([64, F], f32, tag="mol4", bufs=1)
        nc.scalar.activation(m2_sb[:], t_m2[:], RELU)

        wlm = (S32b("wlma"), S32b("wlmb"))
        blm = S32b("blm_col")
        mcol0 = sb.tile([P, 1], f32, tag="mc0", bufs=1)
        mcol1 = sb.tile([P, 1], f32, tag="mc1", bufs=1)
        pool_ps = psM.tile([P, 1], f32, tag="m", space="PSUM")
        nc.tensor.matmul(pool_ps[0:P, :], m2_sb[0:64, 0:P], ones32[0:64, :],
                         start=True, stop=True)
        nc.scalar.activation(mcol0[:], pool_ps[0:P, :], COPY, scale=1.0 / 64.0)
        pool_ps2 = psM.tile([P, 1], f32, tag="m", space="PSUM")
        nc.tensor.matmul(pool_ps2[0:F - P, :], m2_sb[0:64, P:F],
                         ones32[0:64, :], start=True, stop=True)
        nc.scalar.activation(mcol1[0:F - P, :], pool_ps2[0:F - P, :], COPY,
                             scale=1.0 / 64.0)
        mvec = sb.tile([P, 1], f32, tag="mvec", bufs=1)
        mm_ps = psM.tile([P, 1], f32, tag="m", space="PSUM")
        nc.tensor.matmul(mm_ps[:], wlm[0][:], mcol0[:], start=True, stop=False)
        nc.tensor.matmul(mm_ps[:], wlm[1][0:F - P, :], mcol1[0:F - P, :],
                         start=False, stop=True)
        nc.scalar.activation(mvec[:], mm_ps[:], RELU, bias=blm[:])

        # ---- layer 2 -> h3 -> AG3 ----
        layer_tail(agg_blocks(hload(h_full[0])), wc3, bc_rep[1],
                   h_slice[1], ag_halves=h_full[1])
        w1ct = load(wp, "W1ct", eng=nc.gpsimd)     # [128, 12000] f16

        # head weight batch 2
        bl3c = SR("bl3c_row")
        wl3cpk = load(wp, "Wl3c_pk", eng=nc.sync)
        bcat1 = SR("bcat1_row")
        wcat1pk = load(wp, "Wcat1_pk", eng=nc.sync)
        wcat2pk = S32b("wcat2_pk")
        bcat2 = SR("bcat2_t")

        # ---- layer 3 -> z = c3 @ Wc4 [512, 3] -> AG-z ----
        layer_tail(agg_blocks(hload(h_full[1])), wc4, bc_rep[2], z_slice,
                   ncols=3)
        allgather(z_slice[0:CLL_NPC, 0:3], z_full.ap())

        # ---- fusion head minis (fill the AG-z window) ----
        def mm_chain(p_rows, n_cols, k_steps, act_bias, out_tag):
            acc = sb.tile([p_rows, n_cols], f32, tag=out_tag + "a")
            for k in range(k_steps):
                lhsT, rhs = yield k
                pst = psM.tile([p_rows, n_cols], f32, tag="m", space="PSUM")
                for och in range(n_cols):
                    nc.tensor.matmul(pst[:, och:och + 1], lhsT(och), rhs,
                                     start=True, stop=True)
                if k == 0:
                    nc.vector.tensor_copy(acc[:], pst[:])
                else:
                    nc.vector.tensor_tensor(acc[:], acc[:], pst[:], op=ADD)
            o = sb.tile([p_rows, n_cols], f32, tag=out_tag, bufs=1)
            for och in range(n_cols):
                nc.scalar.activation(o[:, och:och + 1], acc[:, och:och + 1],
                                     RELU, bias=act_bias[:, och:och + 1])
            yield o

        def run_chain(p_rows, n_cols, pieces, act_bias, out_tag):
            gen = mm_chain(p_rows, n_cols, len(pieces), act_bias, out_tag)
            k = next(gen)
            while True:
                r = gen.send(pieces[k])
                if not isinstance(r, int):
                    return r
                k = r

        bvec = run_chain(P, 1, [
            (lambda o: wlb[0][:, 0:128], bgc0[:]),
            (lambda o: wlb[1][0:F - P, 0:128], bgc1[0:F - P, :]),
        ], blb, "bvec")

        d1 = run_chain(125, 4, [
            (lambda o: wd1[0][:, o * 125:(o + 1) * 125], mvec[:]),
            (lambda o: wd1[1][:, o * 125:(o + 1) * 125], bvec[:]),
        ], bd1, "d1")

        d2 = run_chain(P, 2, [
            (lambda o, k=k: wd2t[k][:, o * P:(o + 1) * P], d1[:, k:k + 1])
            for k in range(4)
        ], bd2, "d2")

        # ---- P^T = sum_s z_s^T Q_s, h4, dense-1 partials ----
        zf = sb.tile([P, CLL_NCH, 3], f16, tag="zf", bufs=1)
        nc.sync.dma_start(
            zf[:], z_full.ap()[0:CLL_NF, :].rearrange("(c p) f -> p c f", p=P))
        ptz = psM.tile([3, CLL_NPC], f32, tag="m", space="PSUM")
        for s in range(CLL_NCH):
            pi = piece_of(s)
            c0 = (s - PIECES[pi][0]) * CLL_NBLK * P
            nc.tensor.matmul(ptz[:], zf[:, s, 0:3],
                             qt_p[pi][:, c0:c0 + CLL_NBLK * P],
                             start=(s == 0), stop=(s == CLL_NCH - 1))
        ptz_sb = sb.tile([3, CLL_NPC], f32, tag="ptzsb", bufs=1)
        nc.vector.tensor_copy(ptz_sb[:], ptz[:])

        h4c = []
        for b in range(CLL_NBLK):
            pt4 = psT.tile([P, P], f32, tag="tp", space="PSUM")
            nc.tensor.transpose(pt4[0:P, 0:3],
                                ptz_sb[0:3, b * P:(b + 1) * P],
                                ident32[0:3, 0:3])
            th4 = sb.tile([P, 3], f32, tag="th4")
            nc.vector.tensor_tensor(th4[:], pt4[0:P, 0:3], bc4r[:, 0:3],
                                    op=ADD)
            hb = sb.tile([P, 3], f16, tag=f"h4c{b}", bufs=1)
            nc.scalar.activation(hb[:], th4[:], RELU)
            h4c.append(hb)

        # dense-1 partials, block-major so mms start as soon as h4c[0] lands
        JORDER = [ch * 4 + b for b in range(CLL_NBLK) for ch in range(3)]
        dsum = sb.tile([1, 1024], f32, tag="rowb", bufs=1)
        nc.vector.memset(dsum[0:1, 1000:1024], 0.0)
        for half in range(2):
            psd = psM.tile([1, 500], f32, tag="m", space="PSUM")
            for pos, j in enumerate(JORDER):
                ch, b = j // 4, j % 4
                nc.tensor.matmul(psd[:], h4c[b][:, ch:ch + 1],
                                 w1ct[:, j * 1000 + half * 500:
                                      j * 1000 + half * 500 + 500],
                                 start=(pos == 0), stop=(pos == 11))
            nc.vector.tensor_copy(dsum[0:1, half * 500:half * 500 + 500],
                                  psd[0:1, :])
        nc.gpsimd.dma_start(ar_in.ap()[0:1024, None], dsum[0:1, :])

        nc.gpsimd.collective_compute(
            "AllReduce", mybir.AluOpType.add, replica_groups=RG,
            ins=[ar_in.ap()], outs=[ar_out.ap()])

        # ---- fusion head (replicated) ----
        c1in = sb.tile([P, 8], f32, tag="c1in", bufs=1)
        nc.sync.dma_start(c1in[:], ar_out.ap().rearrange("(j p) -> p j", p=P))
        c1t = sb.tile([P, 8], f32, tag="c1t", bufs=1)
        nc.vector.tensor_tensor(c1t[:], c1in[:], bl1c[:], op=ADD)
        c1h = sb.tile([P, 8], f16, tag="c1h", bufs=1)
        nc.scalar.activation(c1h[:], c1t[:], RELU)

        def rowstage(lhs_cols, rhs_pk, rhs_cw, ncols, bias_row, tag,
                     out_f16=True):
            """out_row[1, ncols(+pad)] = relu(sum_j lhs_cols[j]^T rhs_j + b)."""
            npad = max(ncols, 1024) if ncols > 512 else ncols
            rb = sb.tile([1, npad], f32, tag="rowb", bufs=1)
            if npad > ncols:
                nc.vector.memset(rb[0:1, ncols:npad], 0.0)
            for h0 in range(0, ncols, 500):
                hw = min(500, ncols - h0)
                psr = psM.tile([1, hw], f32, tag="m", space="PSUM")
                for j, col in enumerate(lhs_cols):
                    nc.tensor.matmul(psr[:], col,
                                     rhs_pk[:, j * rhs_cw + h0:
                                            j * rhs_cw + h0 + hw],
                                     start=(j == 0),
                                     stop=(j == len(lhs_cols) - 1))
                nc.vector.tensor_tensor(rb[0:1, h0:h0 + hw], psr[0:1, :],
                                        bias_row[0:1, h0:h0 + hw], op=ADD)
            ro = sb.tile([1, npad], f16 if out_f16 else f32, tag=tag + "o",
                         bufs=1)
            nc.scalar.activation(ro[0:1, :], rb[0:1, :], RELU)
            return ro

        def rowcols(row, n, tag, idf):
            cols = sb.tile([P, n], row.dtype, tag=tag, bufs=1)
            for j in range(n):
                ptj = psT.tile([P, P], row.dtype, tag="tp", name="ptj",
                               space="PSUM")
                nc.tensor.transpose(ptj[0:P, 0:1], row[0:1, j * P:(j + 1) * P],
                                    idf[0:1, 0:1])
                nc.vector.tensor_copy(cols[:, j:j + 1], ptj[0:P, 0:1])
            return cols

        c1cols = [c1h[:, j:j + 1] for j in range(8)]
        c2h = rowstage(c1cols, wl2cpk, 1000, 1000, bl2c, "c2")
        c2c = rowcols(c2h, 8, "c2c", ident16)
        c3h = rowstage([c2c[:, j:j + 1] for j in range(8)], wl3cpk, 256, 256,
                       bl3c, "c3")
        d2h = sb.tile([P, 2], f16, tag="d2h", bufs=1)
        nc.vector.tensor_copy(d2h[:], d2[:])
        c3c = rowcols(c3h, 2, "c3c", ident16)
        ucols_in = [d2h[:, 0:1], d2h[:, 1:2], c3c[:, 0:1], c3c[:, 1:2]]
        uact = rowstage(ucols_in, wcat1pk, 1000, 1000, bcat1, "u",
                        out_f16=False)
        ucols = rowcols(uact, 8, "ucols", ident32)
        pso = psM.tile([1, 1], f32, tag="m", space="PSUM")
        for k in range(8):
            nc.tensor.matmul(pso[:], ucols[:, k:k + 1], wcat2pk[:, k:k + 1],
                             start=(k == 0), stop=(k == 7))
        osb = sb.tile([1, 1], f32, tag="osb", bufs=1)
        nc.scalar.activation(osb[:], pso[:], RELU, bias=bcat2[:])
        nc.sync.dma_start(out[0:1, 0:1], osb[:])


# ------------------------------------------------------------------- entry

_CACHE = {}


def kernel(**inputs):
    in_maps, meta = prep_inputs(inputs)
    if "nc" not in _CACHE:
        _CACHE["nc"] = build_program(meta)
    nc = _CACHE["nc"]
    res = run_bass_kernel_spmd(nc, in_maps, core_ids=list(range(NCORES)))
    return np.asarray(res.results[0]["out"], np.float32)
